# revision 15
# baseline (speedup 1.0000x reference)
"""Trainium2 Bass kernel for nn_ExplainerCompatibleGinGru.

Math: the reference pads the batch with 31 zero graphs, splits the node dim
into two 36-node graphs (ad = rows 0:36, dis = rows 36:72), runs 3 GIN layers
with sum-pooling, packs [ad x (L-1), dis] as a GRU sequence per batch
element, and returns out[0] -- which depends ONLY on graph 0 (ad), graph 32
(dis) and L = LOS_batch[0].  So the kernel computes: GIN on the stacked
72-node 2-graph block, an L-step GRU on one sequence, and a tiny classifier.

Runs replicated on all 8 cores (one latency-bound dependency chain;
collectives have a ~5us floor, so no sharding).

Fast path (taken when all GIN/classifier biases are zero and LN gain/bias
are 1/0, as produced by setup_inputs):
- Wih and Whh are stored as pow2-scaled float8e3 (e3m4): LDWEIGHTS streams
  4 B/cycle with FWL vs 2 for fp16 -- these two matrices are ~75% of all
  PE weight-load traffic.  Descales fold into existing ACT scale / DVE
  scalar_tensor_tensor slots, so no extra instructions.
- LN apply + relu fuse into the PSUM evacuation (ACT relu(bias=-m*rstd,
  scale=rstd) / DVE 2-op), so the transpose input is already relu'd.
- The per-layer transpose is a general matmul against [eye72 | graph-masks]:
  the two extra columns produce the sum-pooled features for free inside the
  Wb matmuls' accumulation -- no tensor_reduce pooling pass at all.  The gi
  (Wih) matmuls read their rhs straight out of each layer's hnT tile.
- GRU r+z gates share one PSUM tile -> one bias closer + one sigmoid.
- Single fp16 h state (no parallel f32 copy); shorter gate tail.
- Sqrt and Sigmoid/Tanh live in different ACT table sets: the sigmoid set
  is preloaded via a dummy op while the PE drains gi matmuls, off the
  critical path.

The general path (nonzero biases / LN affine) falls back to the previous
fully-general emission.
"""

import os
import numpy as np
import ml_dtypes

F16 = np.float16
F8E3 = ml_dtypes.float8_e3m4

H = 512
LN_EPS = 1e-5
NSPAN = 74   # 72 node cols + 2 pooled cols per feature chunk

_prog_cache = {}
last_run_info = {}


def _pow2_scale(w, target=12.0):
    am = float(np.abs(w).max())
    if am == 0.0:
        return 1.0
    return float(2.0 ** np.floor(np.log2(target / am)))


def _pack_kchunks_orig(w, ncols):
    """[K, N] weight -> [128, (K//128)*N], chunk kc at cols [N*kc, N*(kc+1))."""
    k, n = w.shape
    assert k % 128 == 0 and n == ncols
    nk = k // 128
    return np.ascontiguousarray(
        w.reshape(nk, 128, n).transpose(1, 0, 2).reshape(128, nk * n))


# ---------------------------------------------------------------- fast path

# sm16 blob layout: name -> (row0, nrows, col0, ncols)
_SLOTS = {}
_SMCOLS = 0


def _slot(name, nrows, ncols):
    global _SMCOLS
    _SLOTS[name] = (0, nrows, _SMCOLS, ncols)
    _SMCOLS += ncols


_slot('bc1t', 8, 128)
_slot('bhhnt', 4, 128)
_slot('eye8', 8, 8)
_slot('wc2', 128, 8)
_slot('eye128', 128, 128)


def _prep_fast(inputs):
    f32 = np.float32

    def bf(x):
        return np.asarray(x, f32).astype(F16)

    x = np.asarray(inputs['x_embedded'], f32)
    tei = np.asarray(inputs['template_edge_index']).astype(np.int64)
    L = int(np.asarray(inputs['LOS_batch']).reshape(-1)[0])

    A = np.zeros((36, 36), f32)
    np.add.at(A, (tei[1], tei[0]), 1.0)
    Mp = A + np.eye(36, dtype=f32)
    m72 = np.zeros((72, 72), f32)
    m72[:36, :36] = Mp.T
    m72[36:, 36:] = Mp.T

    W = {k: np.asarray(v, f32) for k, v in inputs.items()
         if k not in ('x_embedded', 'template_edge_index', 'LOS_batch')}

    sih = _pow2_scale(W['Wih'])
    shh = 1.0  # Whh rides bf16 (FWL-fast LDWEIGHTS), no scaling needed

    # one [128, .] f16 blob carries everything small, in one DMA with fat
    # descriptors (small separate DMAs starve behind the bulk queues):
    # cols 0:584 rows 0:32 = x0T | w1a; cols 584:730 rows 0:72 = m72 |
    # [eye72 | admask | dismask]; cols 730: = the sm16 slots
    vals = {
        'bc1t': W['bc1'].reshape(8, 128),
        'bhhnt': W['bhh'][2 * H:].reshape(4, 128) * shh,
        'eye8': np.eye(8, dtype=f32),
        'wc2': np.ascontiguousarray(W['Wc2'].reshape(8, 128).T),
        'eye128': np.eye(128, dtype=f32),
    }
    sA = np.zeros((32, 584), F16)
    sA[:, 0:72] = bf(x.T)
    sA[:, 72:584] = bf(W['W1a'])
    # bigB (m72 | eye+mask | sm16 slots) leads the gpsimd queue
    big16 = np.zeros((128, 74 + _SMCOLS), F16)
    big16[0:72, 0:72] = bf(m72)
    big16[0:36, 72] = 1.0
    big16[36:72, 73] = 1.0
    for name, (r0, nr, c0, ncn) in _SLOTS.items():
        big16[r0:r0 + nr, 74 + c0:74 + c0 + ncn] = bf(vals[name])

    # f32v: 0:24 combo24 [p, 2j+g] = bih[p+128j] (+ bhh for j<8); 24:28 bhh_n;
    # 28 bc2
    f32v = np.zeros((128, 29), f32)
    bih_t = W['bih'].reshape(12, 128).T
    bhh_t = W['bhh'].reshape(12, 128).T
    combo = bih_t.copy()
    combo[:, 0:8] += bhh_t[:, 0:8]
    f32v[:, 0:24:2] = combo
    f32v[:, 1:24:2] = combo
    f32v[:, 24:28] = bhh_t[:, 8:12]
    f32v[:, 28] = W['bc2'][0]

    gw16 = np.concatenate([
        _pack_kchunks_orig(W['W1b'], H), _pack_kchunks_orig(W['Wha'], H),
        _pack_kchunks_orig(W['Whb'], H)], axis=1).astype(F16)

    def q8(w, s):
        return np.clip(w * s, -15.5, 15.5).astype(F8E3)

    wiht = q8(_pack_kchunks_orig(np.ascontiguousarray(W['Wih'].T), 1536), sih)
    whht = _pack_kchunks_orig(np.ascontiguousarray(W['Whh'].T), 1536).astype(
        ml_dtypes.bfloat16)

    blobs = {
        'sA': sA,
        'big16': big16,
        'f32v': f32v,
        'whht': whht,
        'wc1': bf(_pack_kchunks_orig(W['Wc1'], 1024)),
        'gw16': gw16,
        'wiht0': np.ascontiguousarray(wiht[:, 0:4608]),
        'wiht1': np.ascontiguousarray(wiht[:, 4608:9216]),
        'wiht2': np.ascontiguousarray(wiht[:, 9216:13824]),
        'wiht3': np.ascontiguousarray(wiht[:, 13824:18432]),
    }
    return blobs, L, sih, shh


def _emit_fast(ctx, tc, d, out_dram, L, sih, shh):
    import concourse.mybir as mybir
    nc = tc.nc
    f32 = mybir.dt.float32
    f16 = mybir.dt.float16
    AF = mybir.ActivationFunctionType
    AL = mybir.AluOpType

    wts = ctx.enter_context(tc.tile_pool(name="wts", bufs=1))
    act = ctx.enter_context(tc.tile_pool(name="act", bufs=1))
    tmp = ctx.enter_context(tc.tile_pool(name="tmp", bufs=2))
    pu = ctx.enter_context(tc.tile_pool(name="pu", bufs=2, space="PSUM"))
    pvt = ctx.enter_context(tc.tile_pool(name="pvt", bufs=2, space="PSUM"))
    psm = ctx.enter_context(tc.tile_pool(name="psm", bufs=3, space="PSUM"))
    pgi = ctx.enter_context(tc.tile_pool(name="pgi", bufs=1, space="PSUM"))

    # ---- inputs -> SBUF, ordered by first use across the DMA queues ----
    sA = wts.tile([32, 584], f16, tag='sA')
    nc.sync.dma_start(sA[:, :], d['sA'])
    x0T = sA[:, 0:72]
    w1a = sA[:, 72:584]
    big16 = wts.tile([128, 74 + _SMCOLS], f16, tag='big16')
    nc.gpsimd.dma_start(big16[:, :], d['big16'])
    m72 = big16[0:72, 0:72]
    aggmask = big16[0:72, 0:NSPAN]   # [Mp.T | admask | dismask]
    masks = big16[0:72, 72:74]
    f32v = wts.tile([128, 29], f32, tag='f32v')
    nc.sync.dma_start(f32v[:, :], d['f32v'])
    # wiht3/whht/wc1 are needed late (gi drain/GRU/classifier); their
    # tiles live in thr (bufs=1) behind dummy writers keyed on GIN
    # progress, so their DMAs don't steal early bandwidth from gw16/wiht
    thr = ctx.enter_context(tc.tile_pool(name="thr", bufs=1))
    wiht3_dummy = thr.tile([1, 1], f16, tag='wiht3s')
    whht_dummy = thr.tile([1, 1], f16, tag='whht')
    wc1_dummy = thr.tile([1, 1], f16, tag='wc1')
    wiht3_tile = None

    def S(name):
        r0, nr, c0, ncn = _SLOTS[name]
        return big16[r0:r0 + nr, 74 + c0:74 + c0 + ncn]

    gw16 = wts.tile([128, 3 * 4 * H], f16, tag='gw16')
    wiht_t = [wts.tile([128, 3 * 1536], mybir.dt.float8e3, tag=f'wiht{q}',
                       name=f'wiht{q}') for q in range(3)]
    nc.gpsimd.dma_start(gw16[:, 0:2048], d['gw16'][:, 0:2048])        # w1b
    nc.gpsimd.dma_start(gw16[:, 2048:4096], d['gw16'][:, 2048:4096])  # wha
    nc.gpsimd.dma_start(gw16[:, 4096:6144], d['gw16'][:, 4096:6144])  # whb
    nc.gpsimd.dma_start(wiht_t[0][:, :], d['wiht0'])
    nc.gpsimd.dma_start(wiht_t[1][:, :], d['wiht1'])
    nc.gpsimd.dma_start(wiht_t[2][:, :], d['wiht2'])

    def wiht_chunk(kc, j):
        q, r = divmod(kc, 3)
        base = 1536 * r + 128 * j
        t = wiht3_tile if q == 3 else wiht_t[q]
        return t[:, base:base + 128]

    # prefetch the sqrt ACT table (first LN would otherwise stall ~1.3us)
    sc1 = act.tile([1, 1], f32, tag='sc1')
    nc.vector.memset(sc1[:, :], 1.0)
    sc2 = act.tile([1, 1], f32, tag='sc2')
    eps = act.tile([72, 1], f32, tag='eps')
    nc.vector.memset(eps[:, :], LN_EPS)
    nc.scalar.activation(sc2[:, :], sc1[:, :], AF.Sqrt)

    gi_ps = pgi.tile([128, 24], f32, tag='gi')

    # ---- GIN layers ----------------------------------------------------
    # Activations live feature-major between layers as hnT [128, 4*NSPAN];
    # cols [NSPAN*c, NSPAN*c+72) are nodes, cols +72..74 the pooled feats.
    gi_backlog = []
    hnT_tiles = []
    gi_poffs = []
    hT, hcols = x0T, 32
    for l in range(3):
        wa = w1a if l == 0 else gw16[:, 2048:4096]
        wb = gw16[:, 0:2048] if l == 0 else gw16[:, 4096:6144]
        nk = max(hcols // 128, 1)

        # u = (Mp @ h) @ Wa.  For l>=1, hnT already carries (Mp @ h).T from
        # the previous layer's agg-folded transpose, so u comes straight
        # from those chunks; for l=0, z = x0 @ W1a then u = Mp @ z.
        if l == 0:
            z_h = [pu.tile([72, H // 2], f32, tag='pu', name=f'z{q}')
                   for q in range(2)]
            for q in range(2):
                nc.tensor.matmul(z_h[q][:, :], x0T,
                                 w1a[:, q * (H // 2):(q + 1) * (H // 2)],
                                 start=True, stop=True)
            z_sb = tmp.tile([72, H], f16, tag='z_sb')
            nc.vector.tensor_copy(z_sb[:, 0:H // 2], z_h[0][:, :])
            nc.scalar.copy(z_sb[:, H // 2:], z_h[1][:, :])
            u_h = [pu.tile([72, H // 2], f32, tag='pu', name=f'u{l}{q}')
                   for q in range(2)]
            for q in range(2):
                nc.tensor.matmul(u_h[q][:, :], m72,
                                 z_sb[:, q * (H // 2):(q + 1) * (H // 2)],
                                 start=True, stop=True)
        else:
            u_h = [pu.tile([72, H // 2], f32, tag='pu', name=f'u{l}{q}')
                   for q in range(2)]
            for ci, c in enumerate((0, 2, 1, 3)):
                for q in range(2):
                    nc.tensor.matmul(
                        u_h[q][:, :], hT[c][:, 0:72],
                        wa[:, H * c + q * (H // 2):H * c + (q + 1) * (H // 2)],
                        start=(ci == 0), stop=(ci == 3))

        # interleave layer-1's gi matmuls into layer-3's LN gap (their
        # wiht quarters have landed by then; earlier they'd stall on DMA)
        if l == 2:
            for fn in gi_backlog[:4]:
                fn()
            gi_backlog = gi_backlog[4:]

        # LN stats
        bst = tmp.tile([72, 12], f32, tag='bst')
        nc.vector.bn_stats(bst[:, 0:6], u_h[0][:, :])
        nc.vector.bn_stats(bst[:, 6:12], u_h[1][:, :])
        mv = tmp.tile([72, 2], f32, tag='mv')
        nc.vector.bn_aggr(mv[:, :], bst[:, :])
        negm = tmp.tile([72, 1], f32, tag='negm')
        nc.gpsimd.tensor_scalar(negm[:, :], mv[:, 0:1], -1.0, None, AL.mult)
        std = tmp.tile([72, 1], f32, tag='std')
        nc.scalar.activation(std[:, :], mv[:, 1:2], AF.Sqrt,
                             bias=eps[:, 0:1])
        rstd = tmp.tile([72, 1], f32, tag='rstd')
        nc.vector.reciprocal(rstd[:, :], std[:, :])
        # per-node 1/sigma folds into the transpose rhs, which also carries
        # the NEXT layer's aggregation (Mp.T) and the pooling masks; the
        # last layer only needs the pooled columns
        ncols = NSPAN if l < 2 else 2
        poff = 72 if l < 2 else 0
        dgm = tmp.tile([72, ncols], f16, tag='dgm')
        nc.vector.tensor_scalar_mul(
            dgm[:, :], aggmask if l < 2 else masks, rstd[:, 0:1])

        # r' = relu(u - m) node-major (the 1/sigma rides on dgm; relu
        # commutes with the positive scale); ACT half finishes first, so
        # its chunks (2,3) transpose first
        r_lo = tmp.tile([72, H // 2], f16, tag='r_lo')
        r_hi = tmp.tile([72, H // 2], f16, tag='r_hi')
        nc.scalar.activation(r_hi[:, :], u_h[1][:, :], AF.Relu,
                             bias=negm[:, 0:1], scale=1.0)
        us0 = tmp.tile([72, H // 2], f32, tag='us0')
        nc.vector.tensor_scalar_sub(us0[:, :], u_h[0][:, :], mv[:, 0:1])
        nc.vector.tensor_scalar_max(r_lo[:, :], us0[:, :], 0.0)

        # rT chunks: r'_chunk.T @ (rstd-scaled [Mp.T | masks]) -- the next
        # layer's aggregation and the pooling ride the transpose for free
        rT = tmp.tile([128, 4 * ncols], f16, tag='rT')
        for i, c in enumerate((2, 3, 0, 1)):
            src_r = r_hi if c >= 2 else r_lo
            tp = psm.tile([128, ncols], f32, tag='psm')
            nc.tensor.matmul(tp[:, :],
                             src_r[:, 128 * (c % 2):128 * (c % 2) + 128],
                             dgm, start=True, stop=True)
            if i % 2 == 0:
                nc.vector.tensor_copy(rT[:, ncols * c:ncols * (c + 1)], tp[:, :])
            else:
                nc.scalar.copy(rT[:, ncols * c:ncols * (c + 1)], tp[:, :])

        # vT chunks = Wb-chunk.T @ rT-chunk
        vt_ps = [pvt.tile([128, 2 * ncols], f32, tag='pvt', name=f'vt{l}{q}')
                 for q in range(2)]
        FI = (2, 3, 0, 1)
        for ki, fi in enumerate(FI):
            for fo in range(4):
                q, o = fo % 2, fo // 2
                nc.tensor.matmul(
                    vt_ps[q][:, ncols * o:ncols * (o + 1)],
                    wb[:, H * fi + 128 * fo:H * fi + 128 * fo + 128],
                    rT[:, ncols * fi:ncols * (fi + 1)],
                    start=(ki == 0 and fo < 2), stop=(ki == 3),
                    skip_group_check=True)
        hnT = [act.tile([128, ncols], f16, tag=f'hnT{l}{fo}',
                        name=f'hnT{l}{fo}') for fo in range(4)]
        for fo in range(4):
            q, o = fo % 2, fo // 2
            srcp = vt_ps[q][:, ncols * o:ncols * (o + 1)]
            if fo < 2:
                nc.vector.tensor_copy(hnT[fo][:, :], srcp)
            else:
                nc.scalar.copy(hnT[fo][:, :], srcp)
        hnT_tiles.append(hnT)
        gi_poffs.append(poff)

        # queue this layer's gi matmuls; rhs = the pooled cols of hnT
        def make_gi(lv, kcv, mcv):
            def emit_gi():
                src = hnT_tiles[lv][mcv]
                po = gi_poffs[lv]
                for j in range(12):
                    nc.tensor.matmul(
                        gi_ps[:, 2 * j:2 * j + 2],
                        wiht_chunk(kcv, j),
                        src[:, po:po + 2],
                        start=(kcv == 0 and j == 0), stop=(kcv == 11),
                        skip_group_check=True)
            return emit_gi
        for mc in range(4):
            gi_backlog.append(make_gi(l, 4 * l + mc, mc))
        if l == 0:
            nc.vector.tensor_copy(wiht3_dummy[0:1, 0:1], hnT[0][0:1, 0:1])
            wiht3_tile = thr.tile([128, 3 * 1536], mybir.dt.float8e3,
                                  tag='wiht3s', name='wiht3s')
            nc.sync.dma_start(wiht3_tile[:, :], d['wiht3'])
            nc.vector.tensor_copy(whht_dummy[0:1, 0:1], hnT[0][0:1, 0:1])
            whht = thr.tile([128, 4 * 1536], mybir.dt.bfloat16, tag='whht')
            nc.sync.dma_start(whht[:, :], d['whht'])
        elif l == 2:
            nc.vector.tensor_copy(wc1_dummy[0:1, 0:1], hnT[0][0:1, 0:1])
            wc1 = thr.tile([128, 4 * 1024], f16, tag='wc1')
            nc.sync.dma_start(wc1[:, :], d['wc1'])
        hT = hnT
        hcols = H

    for fn in gi_backlog:
        fn()

    # switch ACT to the sigmoid/tanh table while PE drains gi matmuls;
    # reading layer-3 output pins this after the last LN sqrt
    nc.scalar.activation(sc2[:, :], hnT_tiles[2][0][0:1, 0:1], AF.Sigmoid)

    # ---- GRU setup + step 0 (h=0 so gr=0; gates come from gib2) ----
    gib2 = act.tile([128, 24], f32, tag='gib2')
    nc.vector.scalar_tensor_tensor(gib2[:, :], gi_ps[:, :], 1.0 / sih,
                                   f32v[:, 0:24], AL.mult, AL.add)
    g0 = 0 if L > 1 else 1
    rz = tmp.tile([128, 8], f32, tag='rz')
    nc.scalar.activation(rz[:, :], gib2[:, g0:16:2], AF.Sigmoid)
    nt = tmp.tile([128, 4], f32, tag='nt')
    nc.vector.tensor_tensor(nt[:, :], rz[:, 0:4], f32v[:, 24:28], AL.mult)
    nc.vector.tensor_tensor(nt[:, :], nt[:, :], gib2[:, 16 + g0::2], AL.add)
    n = tmp.tile([128, 4], f32, tag='n')
    nc.scalar.activation(n[:, :], nt[:, :], AF.Tanh)
    w = tmp.tile([128, 4], f32, tag='w')
    nc.vector.tensor_scalar(w[:, :], rz[:, 4:8], -1.0, 1.0, AL.mult, AL.add)
    # h state as two half-tiles so the next step's first matmuls start as
    # soon as the first half lands
    h01 = tmp.tile([128, 2], f16, tag='h01')
    nc.vector.tensor_tensor(h01[:, :], w[:, 0:2], n[:, 0:2], AL.mult)
    h23 = tmp.tile([128, 2], f16, tag='h23')
    nc.vector.tensor_tensor(h23[:, :], w[:, 2:4], n[:, 2:4], AL.mult)

    # r/z bias closers for steps >= 1 (only needed after step 0's gates)
    gibT = []  # per graph: [8, 128] fp16 (r rows 0:4, z rows 4:8)
    for g in range(2):
        gb = tmp.tile([128, 8], f16, tag='gib_h')
        nc.vector.tensor_copy(gb[:, :], gib2[:, g:16:2])
        tp = psm.tile([8, 128], f16, tag='psm')
        nc.tensor.transpose(tp[:, :], gb[:, :], S('eye128'))
        t = act.tile([8, 128], f16, tag=f'gibT{g}')
        nc.scalar.activation(t[:, :], tp[:, :], AF.Identity, scale=shh)
        gibT.append(t)

    def hcol(c):
        return (h01 if c < 2 else h23)[:, c % 2:c % 2 + 1]

    eye4 = S('eye8')[0:4, 0:4]
    for t in range(1, L):
        gs = 0 if t < L - 1 else 1
        rz_ps = psm.tile([128, 8], f32, tag='psm')
        grn = psm.tile([128, 4], f32, tag='psm')
        for j in range(8):
            for c in range(4):
                nc.tensor.matmul(
                    rz_ps[:, j:j + 1],
                    whht[:, 1536 * c + 128 * j:1536 * c + 128 * (j + 1)],
                    hcol(c), start=(c == 0 and j == 0),
                    stop=False, skip_group_check=True)
        nc.tensor.matmul(rz_ps[:, :], gibT[gs][:, :], S('eye8'),
                         start=False, stop=True, skip_group_check=True)
        for j in range(8, 12):
            for c in range(4):
                nc.tensor.matmul(
                    grn[:, j - 8:j - 7],
                    whht[:, 1536 * c + 128 * j:1536 * c + 128 * (j + 1)],
                    hcol(c), start=(c == 0 and j == 8),
                    stop=False, skip_group_check=True)
        nc.tensor.matmul(grn[:, :], S('bhhnt'), eye4,
                         start=False, stop=True, skip_group_check=True)

        rzs = tmp.tile([128, 8], f32, tag='rzs')
        nc.scalar.activation(rzs[:, :], rz_ps[:, :], AF.Sigmoid,
                             scale=1.0 / shh)
        zh = tmp.tile([128, 4], f32, tag='zh')
        nc.vector.tensor_tensor(zh[:, 0:2], rzs[:, 4:6], h01[:, :], AL.mult)
        nc.vector.tensor_tensor(zh[:, 2:4], rzs[:, 6:8], h23[:, :], AL.mult)
        w = tmp.tile([128, 4], f32, tag='w')
        nc.gpsimd.tensor_scalar(w[:, :], rzs[:, 4:8], -1.0, 1.0,
                                AL.mult, AL.add)
        nt = tmp.tile([128, 4], f32, tag='nt')
        nc.vector.scalar_tensor_tensor(nt[:, :], grn[:, :], 1.0 / shh,
                                       rzs[:, 0:4], AL.mult, AL.mult)
        nc.vector.tensor_tensor(nt[:, :], nt[:, :], gib2[:, 16 + gs::2],
                                AL.add)
        n = tmp.tile([128, 4], f32, tag='n')
        nc.scalar.activation(n[:, :], nt[:, :], AF.Tanh)
        wn = tmp.tile([128, 4], f32, tag='wn')
        nc.vector.tensor_tensor(wn[:, :], w[:, :], n[:, :], AL.mult)
        h01 = tmp.tile([128, 2], f16, tag='h01')
        nc.vector.tensor_tensor(h01[:, :], wn[:, 0:2], zh[:, 0:2], AL.add)
        h23 = tmp.tile([128, 2], f16, tag='h23')
        nc.gpsimd.tensor_tensor(h23[:, :], wn[:, 2:4], zh[:, 2:4], AL.add)

    # ---- classifier ----
    hid_ps = psm.tile([128, 8], f32, tag='psm')
    for mc in range(8):
        for c in range(4):
            nc.tensor.matmul(
                hid_ps[:, mc:mc + 1],
                wc1[:, 1024 * c + 128 * mc:1024 * c + 128 * (mc + 1)],
                hcol(c), start=(c == 0 and mc == 0), stop=False,
                skip_group_check=True)
    nc.tensor.matmul(hid_ps[:, :], S('bc1t'), S('eye8'),
                     start=False, stop=True, skip_group_check=True)
    hid = tmp.tile([128, 8], f16, tag='hid_sb')
    nc.scalar.activation(hid[:, :], hid_ps[:, :], AF.Relu)
    fin_ps = psm.tile([1, 1], f32, tag='psm')
    for mc in range(8):
        nc.tensor.matmul(fin_ps[:, :], hid[:, mc:mc + 1], S('wc2')[:, mc:mc + 1],
                         start=(mc == 0), stop=(mc == 7))
    out_sb = tmp.tile([1, 1], f32, tag='out_sb')
    nc.scalar.activation(out_sb[:, :], fin_ps[:, :], AF.Identity,
                         bias=f32v[0:1, 28:29], scale=1.0)
    nc.sync.dma_start(out_dram, out_sb[:, :], single_packet=True)


# ------------------------------------------------------------ general path
# (previous fully-general emission; used when biases/LN affine are nonzero)

_XSLOTS = {}
_XCOLS = 0


def _xslot(name, nrows, ncols):
    global _XCOLS
    _XSLOTS[name] = (nrows, _XCOLS, ncols)
    _XCOLS += ncols


_xslot('x0', 72, 32)
_xslot('eye72', 72, 72)
_xslot('w1a', 32, H)
_xslot('m72', 72, 72)
_xslot('ones72', 1, 72)
_xslot('brows', 1, 4 * H + 2)

_GSLOTS = {}
_GSMCOLS = 0


def _gslot(name, nrows, ncols):
    global _GSMCOLS
    _GSLOTS[name] = (0, nrows, _GSMCOLS, ncols)
    _GSMCOLS += ncols


_gslot('bc1t', 8, 128)
_gslot('bhhnt', 4, 128)
_gslot('eye8', 8, 8)
_gslot('wc2', 128, 8)
_gslot('eye128', 128, 128)


def _prep_general(inputs):
    f32 = np.float32

    def bf(x):
        return np.asarray(x, f32).astype(F16)

    x = np.asarray(inputs['x_embedded'], f32)
    tei = np.asarray(inputs['template_edge_index']).astype(np.int64)
    L = int(np.asarray(inputs['LOS_batch']).reshape(-1)[0])

    A = np.zeros((36, 36), f32)
    np.add.at(A, (tei[1], tei[0]), 1.0)
    Mp = A + np.eye(36, dtype=f32)
    m72 = np.zeros((72, 72), f32)
    m72[:36, :36] = Mp.T
    m72[36:, 36:] = Mp.T

    W = {k: np.asarray(v, f32) for k, v in inputs.items()
         if k not in ('x_embedded', 'template_edge_index', 'LOS_batch')}

    xvals = {
        'x0': x,
        'eye72': np.eye(72, dtype=f32),
        'w1a': W['W1a'],
        'm72': m72,
        'ones72': np.ones((1, 72), f32),
        'brows': np.concatenate(
            [W['b1a'], W['b1b'], W['bha'], W['bhb'], [0.0], [0.0]]
        ).reshape(1, 4 * H + 2),
    }
    xe16 = np.zeros((72, _XCOLS), F16)
    for name, (nr, c0, ncn) in _XSLOTS.items():
        xe16[0:nr, c0:c0 + ncn] = bf(xvals[name])

    vals = {
        'bc1t': W['bc1'].reshape(8, 128),
        'bhhnt': W['bhh'][2 * H:].reshape(4, 128),
        'eye8': np.eye(8, dtype=f32),
        'wc2': np.ascontiguousarray(W['Wc2'].reshape(8, 128).T),
        'eye128': np.eye(128, dtype=f32),
    }
    sm16 = np.zeros((128, _GSMCOLS), F16)
    for name, (r0, nr, c0, ncn) in _GSLOTS.items():
        sm16[r0:r0 + nr, c0:c0 + ncn] = bf(vals[name])

    f32v = np.zeros((128, 53), f32)
    bih_t = W['bih'].reshape(12, 128).T
    bhh_t = W['bhh'].reshape(12, 128).T
    combo = bih_t.copy()
    combo[:, 0:8] += bhh_t[:, 0:8]
    f32v[:, 0:24:2] = combo
    f32v[:, 1:24:2] = combo
    f32v[:, 24:28] = bhh_t[:, 8:12]
    f32v[:, 28] = W['bc2'][0]
    f32v[:, 29:33] = W['g1'].reshape(4, 128).T
    f32v[:, 33:37] = W['be1'].reshape(4, 128).T
    f32v[:, 37:41] = W['gh'].reshape(4, 128).T
    f32v[:, 41:45] = W['beh'].reshape(4, 128).T
    f32v[:, 45:49] = W['b1b'].reshape(4, 128).T
    f32v[:, 49:53] = W['bhb'].reshape(4, 128).T

    gw16 = np.concatenate([
        _pack_kchunks_orig(W['W1b'], H), _pack_kchunks_orig(W['Wha'], H),
        _pack_kchunks_orig(W['Whb'], H)], axis=1).astype(F16)

    blobs = {
        'xe0': xe16[:, 0:104].copy(),
        'xe16': xe16,
        'sm16': sm16,
        'gw16': gw16,
        'f32v': f32v,
        'wiht': bf(_pack_kchunks_orig(np.ascontiguousarray(W['Wih'].T), 1536)),
        'whht': bf(_pack_kchunks_orig(np.ascontiguousarray(W['Whh'].T), 1536)),
        'wc1': bf(_pack_kchunks_orig(W['Wc1'], 1024)),
    }
    return blobs, L


def _emit_general(ctx, tc, d, out_dram, L):
    import concourse.mybir as mybir
    nc = tc.nc
    f32 = mybir.dt.float32
    f16 = mybir.dt.float16
    AF = mybir.ActivationFunctionType
    AL = mybir.AluOpType

    wts = ctx.enter_context(tc.tile_pool(name="wts", bufs=1))
    act = ctx.enter_context(tc.tile_pool(name="act", bufs=1))
    tmp = ctx.enter_context(tc.tile_pool(name="tmp", bufs=2))
    pu = ctx.enter_context(tc.tile_pool(name="pu", bufs=2, space="PSUM"))
    pvt = ctx.enter_context(tc.tile_pool(name="pvt", bufs=2, space="PSUM"))
    psm = ctx.enter_context(tc.tile_pool(name="psm", bufs=3, space="PSUM"))
    pgi = ctx.enter_context(tc.tile_pool(name="pgi", bufs=1, space="PSUM"))

    xe0 = wts.tile([72, 104], f16, tag='xe0')
    nc.sync.dma_start(xe0[:, :], d['xe0'])
    x0s = xe0[:, 0:32]
    eye72 = xe0[:, 32:104]
    xe16 = wts.tile([72, _XCOLS], f16, tag='xe16')
    nc.sync.dma_start(xe16[:, :], d['xe16'])

    def X(name):
        nr, c0, ncn = _XSLOTS[name]
        return xe16[0:nr, c0:c0 + ncn]

    sm16 = wts.tile([128, _GSMCOLS], f16, tag='sm16')
    nc.sync.dma_start(sm16[:, :], d['sm16'])
    f32v = wts.tile([128, 53], f32, tag='f32v')
    nc.sync.dma_start(f32v[:, :], d['f32v'])
    whht = wts.tile([128, 4 * 1536], f16, tag='whht')
    nc.sync.dma_start(whht[:, :], d['whht'])
    wc1 = wts.tile([128, 4 * 1024], f16, tag='wc1')
    nc.sync.dma_start(wc1[:, :], d['wc1'])

    def S(name):
        r0, nr, c0, ncn = _GSLOTS[name]
        return sm16[r0:r0 + nr, c0:c0 + ncn]

    gw16 = wts.tile([128, 3 * 4 * H], f16, tag='gw16')
    wiht_t = [wts.tile([128, 3 * 1536], f16, tag=f'wiht{q}',
                       name=f'wiht{q}') for q in range(4)]
    nc.gpsimd.dma_start(gw16[:, 0:2048], d['gw16'][:, 0:2048])
    nc.gpsimd.dma_start(wiht_t[0][:, :], d['wiht'][:, 0:4608])
    nc.gpsimd.dma_start(gw16[:, 2048:4096], d['gw16'][:, 2048:4096])
    nc.gpsimd.dma_start(wiht_t[1][:, :], d['wiht'][:, 4608:9216])
    nc.gpsimd.dma_start(gw16[:, 4096:6144], d['gw16'][:, 4096:6144])
    nc.gpsimd.dma_start(wiht_t[2][:, :], d['wiht'][:, 9216:13824])
    nc.gpsimd.dma_start(wiht_t[3][:, :], d['wiht'][:, 13824:18432])

    def wiht_chunk(kc, j):
        q, r = divmod(kc, 3)
        base = 1536 * r + 128 * j
        return wiht_t[q][:, base:base + 128]

    sc1 = act.tile([1, 1], f32, tag='sc1')
    nc.vector.memset(sc1[:, :], 1.0)
    sc2 = act.tile([1, 1], f32, tag='sc2')
    eps = act.tile([72, 1], f32, tag='eps')
    nc.vector.memset(eps[:, :], LN_EPS)
    nc.scalar.activation(sc2[:, :], sc1[:, :], AF.Sqrt)

    featsT = act.tile([128, 24], f16, tag='featsT')
    gi_ps = pgi.tile([128, 24], f32, tag='gi')

    x0T = tmp.tile([32, 72], f16, tag='x0T')
    tp0 = psm.tile([128, 72], f16, tag='psm')
    nc.tensor.transpose(tp0[0:32, :], x0s, eye72)
    nc.vector.tensor_copy(x0T[:, :], tp0[0:32, :])

    gi_backlog = []
    hT = x0T
    hcols = 32
    for l in range(3):
        wa = X('w1a') if l == 0 else gw16[:, 2048:4096]
        wb = gw16[:, 0:2048] if l == 0 else gw16[:, 4096:6144]
        ba_off = 0 if l == 0 else 2 * H
        gcol = 29 if l == 0 else 37
        becol = 33 if l == 0 else 41
        bbtcol = 45 if l == 0 else 49
        nk = max(hcols // 128, 1)

        z_h = [pbig.tile([72, H // 2], f32, tag='pbig', name=f'z{q}')
               for q in range(2)]
        for c in range(nk):
            cs = min(128, hcols - 128 * c)
            for q in range(2):
                rhs = (wa if l == 0 else wa[:, H * c:H * (c + 1)])[
                    :, q * (H // 2):(q + 1) * (H // 2)]
                nc.tensor.matmul(z_h[q][:, :],
                                 hT[0:cs, 72 * c:72 * (c + 1)], rhs,
                                 start=(c == 0), stop=(c == nk - 1))
        z_sb = tmp.tile([72, H], f16, tag='z_sb')
        nc.vector.tensor_copy(z_sb[:, 0:H // 2], z_h[0][:, :])
        nc.scalar.copy(z_sb[:, H // 2:], z_h[1][:, :])

        u_h = [pbig.tile([72, H // 2], f32, tag='pbig', name=f'u{q}')
               for q in range(2)]
        for q in range(2):
            nc.tensor.matmul(u_h[q][:, :], X('m72'),
                             z_sb[:, q * (H // 2):(q + 1) * (H // 2)],
                             start=True, stop=False)
        for q in range(2):
            off = ba_off + q * (H // 2)
            nc.tensor.matmul(u_h[q][:, :], X('ones72'),
                             X('brows')[:, off:off + H // 2],
                             start=False, stop=True)

        bst = tmp.tile([72, 12], f32, tag='bst')
        nc.vector.bn_stats(bst[:, 0:6], u_h[0][:, :])
        nc.vector.bn_stats(bst[:, 6:12], u_h[1][:, :])
        mv = tmp.tile([72, 2], f32, tag='mv')
        nc.vector.bn_aggr(mv[:, :], bst[:, :])
        std = tmp.tile([72, 1], f32, tag='std')
        nc.scalar.activation(std[:, :], mv[:, 1:2], AF.Sqrt,
                             bias=eps[:, 0:1])
        rstd = tmp.tile([72, 1], f32, tag='rstd')
        nc.vector.reciprocal(rstd[:, :], std[:, :])
        mb = tmp.tile([72, 1], f32, tag='mb')
        nc.vector.scalar_tensor_tensor(mb[:, :], mv[:, 0:1], -1.0,
                                       rstd[:, 0:1], AL.mult, AL.mult)

        us = tmp.tile([72, H], f16, tag='us')
        nc.vector.tensor_scalar(us[:, 0:H // 2], u_h[0][:, :],
                                mv[:, 0:1], rstd[:, 0:1],
                                AL.subtract, AL.mult)
        nc.scalar.activation(us[:, H // 2:], u_h[1][:, :], AF.Identity,
                             bias=mb[:, 0:1], scale=rstd[:, 0:1])
        rT = tmp.tile([128, 4 * 72], f16, tag='rT')
        for c in range(4):
            tp = psm.tile([128, 72], f16, tag='psm')
            nc.tensor.transpose(tp[:, :], us[:, 128 * c:128 * (c + 1)],
                                eye72)
            nc.scalar.activation(rT[:, 72 * c:72 * (c + 1)], tp[:, :], AF.Relu,
                                 bias=f32v[:, becol + c:becol + c + 1],
                                 scale=f32v[:, gcol + c:gcol + c + 1])

        vt_ps = [pbig.tile([128, 2 * 72], f32, tag='pvt', name=f'vt{q}')
                 for q in range(2)]
        for fi in range(4):
            for fo in range(4):
                q, o = fo % 2, fo // 2
                nc.tensor.matmul(
                    vt_ps[q][:, 72 * o:72 * (o + 1)],
                    wb[:, H * fi + 128 * fo:H * fi + 128 * fo + 128],
                    rT[:, 72 * fi:72 * (fi + 1)],
                    start=(fi == 0 and fo < 2), stop=(fi == 3),
                    skip_group_check=True)
        hnT = tmp.tile([128, 4 * 72], f16, tag='hnT')
        for fo in range(4):
            q, o = fo % 2, fo // 2
            dst = hnT[:, 72 * fo:72 * (fo + 1)]
            srcp = vt_ps[q][:, 72 * o:72 * (o + 1)]
            bb = f32v[:, bbtcol + fo:bbtcol + fo + 1]
            if fo < 2:
                nc.vector.tensor_scalar_add(dst, srcp, bb[:, 0:1])
            else:
                nc.scalar.activation(dst, srcp, AF.Identity, bias=bb[:, 0:1])

        pf = tmp.tile([128, 8], f32, tag='pf')
        for fo in range(4):
            for g in range(2):
                nc.vector.tensor_reduce(
                    pf[:, 2 * fo + g:2 * fo + g + 1],
                    hnT[:, 72 * fo + 36 * g:72 * fo + 36 * g + 36],
                    mybir.AxisListType.X, AL.add)
        nc.vector.tensor_copy(featsT[:, 8 * l:8 * l + 8], pf[:, :])

        def make_gi(kcv):
            def emit_gi():
                for j in range(12):
                    nc.tensor.matmul(
                        gi_ps[:, 2 * j:2 * j + 2],
                        wiht_chunk(kcv, j),
                        featsT[:, 2 * kcv:2 * kcv + 2],
                        start=(kcv == 0 and j == 0), stop=(kcv == 11),
                        skip_group_check=True)
            return emit_gi
        for mc in range(4):
            gi_backlog.append(make_gi(4 * l + mc))
        hT = hnT
        hcols = H

    for kc in range(12):
        gi_backlog[kc]()
    gi_backlog = []

    gib2 = act.tile([128, 24], f32, tag='gib2')
    nc.vector.tensor_tensor(gib2[:, :], gi_ps[:, :], f32v[:, 0:24], AL.add)
    gibT = []
    for g in range(2):
        pair = []
        for half in range(2):
            gb = tmp.tile([128, 4], f16, tag='gib_h')
            nc.vector.tensor_copy(gb[:, :], gib2[:, g + 8 * half:g + 8 * half + 8:2])
            tp = psm.tile([4, 128], f16, tag='psm')
            nc.tensor.transpose(tp[:, :], gb[:, :], S('eye128'))
            t = act.tile([4, 128], f16, tag=f'gibT{g}{half}')
            nc.vector.tensor_copy(t[:, :], tp[:, :])
            pair.append(t)
        gibT.append(pair)

    g0 = 0 if L > 1 else 1
    rz = tmp.tile([128, 8], f32, tag='rz')
    nc.scalar.activation(rz[:, :], gib2[:, g0:16:2], AF.Sigmoid)
    nt = tmp.tile([128, 4], f32, tag='nt')
    nc.vector.tensor_tensor(nt[:, :], rz[:, 0:4], f32v[:, 24:28], AL.mult)
    nc.vector.tensor_tensor(nt[:, :], nt[:, :], gib2[:, 16 + g0::2], AL.add)
    n = tmp.tile([128, 4], f32, tag='n')
    nc.scalar.activation(n[:, :], nt[:, :], AF.Tanh)
    w = tmp.tile([128, 4], f32, tag='w')
    nc.gpsimd.tensor_scalar(w[:, :], rz[:, 4:8], -1.0, 1.0, AL.mult, AL.add)
    h_f = tmp.tile([128, 4], f32, tag='h_f')
    nc.gpsimd.tensor_tensor(h_f[:, :], w[:, :], n[:, :], AL.mult)
    h_b = tmp.tile([128, 4], f16, tag='h_b')
    nc.vector.tensor_tensor(h_b[:, :], w[:, :], n[:, :], AL.mult)

    eye4 = S('eye8')[0:4, 0:4]
    for t in range(1, L):
        gs = 0 if t < L - 1 else 1
        grr = psm.tile([128, 4], f32, tag='psm')
        grn = psm.tile([128, 4], f32, tag='psm')
        grz = psm.tile([128, 4], f32, tag='psm')
        for out_ps, js, closer in (
                (grr, range(0, 4), (gibT[gs][0][:, :], eye4)),
                (grn, range(8, 12), (S('bhhnt'), eye4)),
                (grz, range(4, 8), (gibT[gs][1][:, :], eye4))):
            j0 = js[0]
            for j in js:
                for c in range(4):
                    nc.tensor.matmul(
                        out_ps[:, j - j0:j - j0 + 1],
                        whht[:, 1536 * c + 128 * j:1536 * c + 128 * (j + 1)],
                        h_b[:, c:c + 1], start=(c == 0 and j == j0),
                        stop=False, skip_group_check=True)
            nc.tensor.matmul(out_ps[:, :], closer[0], closer[1],
                             start=False, stop=True, skip_group_check=True)

        r = tmp.tile([128, 4], f32, tag='r')
        nc.scalar.activation(r[:, :], grr[:, :], AF.Sigmoid)
        nt = tmp.tile([128, 4], f32, tag='nt')
        nc.vector.tensor_tensor(nt[:, :], r[:, :], grn[:, :], AL.mult)
        nc.vector.tensor_tensor(nt[:, :], nt[:, :], gib2[:, 16 + gs::2],
                                AL.add)
        n = tmp.tile([128, 4], f32, tag='n')
        nc.scalar.activation(n[:, :], nt[:, :], AF.Tanh)
        z = tmp.tile([128, 4], f32, tag='z')
        nc.scalar.activation(z[:, :], grz[:, :], AF.Sigmoid)
        zh = tmp.tile([128, 4], f32, tag='zh')
        nc.gpsimd.tensor_tensor(zh[:, :], z[:, :], h_f[:, :], AL.mult)
        w = tmp.tile([128, 4], f32, tag='w')
        nc.gpsimd.tensor_scalar(w[:, :], z[:, :], -1.0, 1.0, AL.mult, AL.add)
        wn = tmp.tile([128, 4], f32, tag='wn')
        nc.vector.tensor_tensor(wn[:, :], w[:, :], n[:, :], AL.mult)
        h_f = tmp.tile([128, 4], f32, tag='h_f')
        nc.gpsimd.tensor_tensor(h_f[:, :], wn[:, :], zh[:, :], AL.add)
        h_b = tmp.tile([128, 4], f16, tag='h_b')
        nc.vector.tensor_tensor(h_b[:, :], wn[:, :], zh[:, :], AL.add)

    hid_ps = psm.tile([128, 8], f32, tag='psm')
    for mc in range(8):
        for c in range(4):
            nc.tensor.matmul(
                hid_ps[:, mc:mc + 1],
                wc1[:, 1024 * c + 128 * mc:1024 * c + 128 * (mc + 1)],
                h_b[:, c:c + 1], start=(c == 0 and mc == 0), stop=False,
                skip_group_check=True)
    nc.tensor.matmul(hid_ps[:, :], S('bc1t'), S('eye8'),
                     start=False, stop=True, skip_group_check=True)
    hid = tmp.tile([128, 8], f16, tag='hid_sb')
    nc.scalar.activation(hid[:, :], hid_ps[:, :], AF.Relu)
    fin_ps = psm.tile([1, 1], f32, tag='psm')
    for mc in range(8):
        nc.tensor.matmul(fin_ps[:, :], hid[:, mc:mc + 1], S('wc2')[:, mc:mc + 1],
                         start=(mc == 0), stop=(mc == 7))
    out_sb = tmp.tile([1, 1], f32, tag='out_sb')
    nc.scalar.activation(out_sb[:, :], fin_ps[:, :], AF.Identity,
                         bias=f32v[0:1, 28:29], scale=1.0)
    nc.sync.dma_start(out_dram, out_sb[:, :])


# --------------------------------------------------------------- plumbing

def _is_fast_path(inputs):
    f32 = np.float32
    zeros = ['b1a', 'be1', 'b1b', 'bha', 'beh', 'bhb']
    ones = ['g1', 'gh']
    for k in zeros:
        if np.any(np.asarray(inputs[k], f32) != 0.0):
            return False
    for k in ones:
        if np.any(np.asarray(inputs[k], f32) != 1.0):
            return False
    return True


def _build_program(key, blobs, emit_fn):
    from contextlib import ExitStack
    import concourse.bacc as bacc
    import concourse.tile as tile
    import concourse.mybir as mybir

    nc = bacc.Bacc("TRN2", target_bir_lowering=False, debug=False,
                   num_devices=8)
    d = {}
    for name, arr in blobs.items():
        d[name] = nc.dram_tensor(name, list(arr.shape),
                                 mybir.dt.from_np(arr.dtype),
                                 kind="ExternalInput").ap()
    out_dram = nc.dram_tensor("out", [1], mybir.dt.float32,
                              kind="ExternalOutput").ap()
    with tile.TileContext(nc) as tc:
        with ExitStack() as ctx:
            emit_fn(ctx, tc, d, out_dram)
    nc.compile()
    return nc


def _install_ntff_hook():
    """The agent image's antenv lacks axon_hooks; recreate it so
    run_bass_kernel_spmd(trace=True) can capture NTFF profiles."""
    import sys, types
    try:
        import antenv
        if 'antenv.axon_hooks' in sys.modules:
            return
        mod = types.ModuleType('antenv.axon_hooks')
        mod._hook = None

        def set_axon_ntff_profile_hook(hk):
            mod._hook = hk

        def get_axon_ntff_profile_hook():
            return mod._hook

        mod.set_axon_ntff_profile_hook = set_axon_ntff_profile_hook
        mod.get_axon_ntff_profile_hook = get_axon_ntff_profile_hook
        sys.modules['antenv.axon_hooks'] = mod
        antenv.axon_hooks = mod
        from trn_agent_boot.trn_boot import _ntff_profile_via_ctypes
        so = '/opt/axon/libaxon_pjrt.so'
        if os.path.exists(so):
            mod._hook = _ntff_profile_via_ctypes(so)
    except Exception as e:  # profiling is best-effort
        print(f"ntff hook install failed: {e}")


def kernel(**inputs):
    from concourse.bass_utils import run_bass_kernel_spmd

    fast = _is_fast_path(inputs)
    if fast:
        blobs, L, sih, shh = _prep_fast(inputs)
        key = ('fast', L)
        if key not in _prog_cache:
            _prog_cache[key] = _build_program(
                key, blobs,
                lambda ctx, tc, d, o: _emit_fast(ctx, tc, d, o, L, sih, shh))
    else:
        blobs, L = _prep_general(inputs)
        key = ('gen', L)
        if key not in _prog_cache:
            _prog_cache[key] = _build_program(
                key, blobs,
                lambda ctx, tc, d, o: _emit_general(ctx, tc, d, o, L))
    nc = _prog_cache[key]

    in_maps = [dict(blobs) for _ in range(8)]
    trace = bool(int(os.environ.get('KERNEL_TRACE', '0')))
    if trace:
        _install_ntff_hook()
    res = run_bass_kernel_spmd(nc, in_maps, list(range(8)), trace=trace)
    last_run_info['exec_time_ns'] = res.exec_time_ns
    last_run_info['results'] = res
    return np.asarray(res.results[0]['out'], np.float32).reshape(1)


# revision 16
# speedup vs baseline: 1.0153x; 1.0153x over previous
"""Trainium2 Bass kernel for nn_ExplainerCompatibleGinGru.

Math: the reference pads the batch with 31 zero graphs, splits the node dim
into two 36-node graphs (ad = rows 0:36, dis = rows 36:72), runs 3 GIN layers
with sum-pooling, packs [ad x (L-1), dis] as a GRU sequence per batch
element, and returns out[0] -- which depends ONLY on graph 0 (ad), graph 32
(dis) and L = LOS_batch[0].  So the kernel computes: GIN on the stacked
72-node 2-graph block, an L-step GRU on one sequence, and a tiny classifier.

Runs replicated on all 8 cores (one latency-bound dependency chain;
collectives have a ~5us floor, so no sharding).

Fast path (taken when all GIN/classifier biases are zero and LN gain/bias
are 1/0, as produced by setup_inputs):
- Wih and Whh are stored as pow2-scaled float8e3 (e3m4): LDWEIGHTS streams
  4 B/cycle with FWL vs 2 for fp16 -- these two matrices are ~75% of all
  PE weight-load traffic.  Descales fold into existing ACT scale / DVE
  scalar_tensor_tensor slots, so no extra instructions.
- LN apply + relu fuse into the PSUM evacuation (ACT relu(bias=-m*rstd,
  scale=rstd) / DVE 2-op), so the transpose input is already relu'd.
- The per-layer transpose is a general matmul against [eye72 | graph-masks]:
  the two extra columns produce the sum-pooled features for free inside the
  Wb matmuls' accumulation -- no tensor_reduce pooling pass at all.  The gi
  (Wih) matmuls read their rhs straight out of each layer's hnT tile.
- GRU r+z gates share one PSUM tile -> one bias closer + one sigmoid.
- Single fp16 h state (no parallel f32 copy); shorter gate tail.
- Sqrt and Sigmoid/Tanh live in different ACT table sets: the sigmoid set
  is preloaded via a dummy op while the PE drains gi matmuls, off the
  critical path.

The general path (nonzero biases / LN affine) falls back to the previous
fully-general emission.
"""

import os
import numpy as np
import ml_dtypes

F16 = np.float16
F8E3 = ml_dtypes.float8_e3m4

H = 512
LN_EPS = 1e-5
NSPAN = 74   # 72 node cols + 2 pooled cols per feature chunk

_prog_cache = {}
last_run_info = {}


def _pow2_scale(w, target=12.0):
    am = float(np.abs(w).max())
    if am == 0.0:
        return 1.0
    return float(2.0 ** np.floor(np.log2(target / am)))


def _pack_kchunks_orig(w, ncols):
    """[K, N] weight -> [128, (K//128)*N], chunk kc at cols [N*kc, N*(kc+1))."""
    k, n = w.shape
    assert k % 128 == 0 and n == ncols
    nk = k // 128
    return np.ascontiguousarray(
        w.reshape(nk, 128, n).transpose(1, 0, 2).reshape(128, nk * n))


# ---------------------------------------------------------------- fast path

# sm16 blob layout: name -> (row0, nrows, col0, ncols)
_SLOTS = {}
_SMCOLS = 0


def _slot(name, nrows, ncols):
    global _SMCOLS
    _SLOTS[name] = (0, nrows, _SMCOLS, ncols)
    _SMCOLS += ncols


_slot('bc1t', 8, 128)
_slot('bhhnt', 4, 128)
_slot('eye8', 8, 8)
_slot('wc2', 128, 8)
_slot('eye128', 128, 128)


def _prep_fast(inputs):
    f32 = np.float32

    def bf(x):
        return np.asarray(x, f32).astype(F16)

    x = np.asarray(inputs['x_embedded'], f32)
    tei = np.asarray(inputs['template_edge_index']).astype(np.int64)
    L = int(np.asarray(inputs['LOS_batch']).reshape(-1)[0])

    A = np.zeros((36, 36), f32)
    np.add.at(A, (tei[1], tei[0]), 1.0)
    Mp = A + np.eye(36, dtype=f32)
    m72 = np.zeros((72, 72), f32)
    m72[:36, :36] = Mp.T
    m72[36:, 36:] = Mp.T

    W = {k: np.asarray(v, f32) for k, v in inputs.items()
         if k not in ('x_embedded', 'template_edge_index', 'LOS_batch')}

    sih = _pow2_scale(W['Wih'])
    shh = 1.0  # Whh rides bf16 (FWL-fast LDWEIGHTS), no scaling needed

    # one [128, .] f16 blob carries everything small, in one DMA with fat
    # descriptors (small separate DMAs starve behind the bulk queues):
    # cols 0:584 rows 0:32 = x0T | w1a; cols 584:730 rows 0:72 = m72 |
    # [eye72 | admask | dismask]; cols 730: = the sm16 slots
    vals = {
        'bc1t': W['bc1'].reshape(8, 128),
        'bhhnt': W['bhh'][2 * H:].reshape(4, 128) * shh,
        'eye8': np.eye(8, dtype=f32),
        'wc2': np.ascontiguousarray(W['Wc2'].reshape(8, 128).T),
        'eye128': np.eye(128, dtype=f32),
    }
    sA = np.zeros((32, 584), F16)
    sA[:, 0:72] = bf(x.T)
    sA[:, 72:584] = bf(W['W1a'])
    # bigB (m72 | eye+mask | sm16 slots) leads the gpsimd queue
    big16 = np.zeros((128, 74 + _SMCOLS), F16)
    big16[0:72, 0:72] = bf(m72)
    big16[0:36, 72] = 1.0
    big16[36:72, 73] = 1.0
    for name, (r0, nr, c0, ncn) in _SLOTS.items():
        big16[r0:r0 + nr, 74 + c0:74 + c0 + ncn] = bf(vals[name])

    # f32v: 0:24 combo24 [p, 2j+g] = bih[p+128j] (+ bhh for j<8); 24:28 bhh_n;
    # 28 bc2
    f32v = np.zeros((128, 29), f32)
    bih_t = W['bih'].reshape(12, 128).T
    bhh_t = W['bhh'].reshape(12, 128).T
    combo = bih_t.copy()
    combo[:, 0:8] += bhh_t[:, 0:8]
    f32v[:, 0:24:2] = combo
    f32v[:, 1:24:2] = combo
    f32v[:, 24:28] = bhh_t[:, 8:12]
    f32v[:, 28] = W['bc2'][0]

    gw16 = np.concatenate([
        _pack_kchunks_orig(W['W1b'], H), _pack_kchunks_orig(W['Wha'], H),
        _pack_kchunks_orig(W['Whb'], H)], axis=1).astype(F16)

    def q8(w, s):
        return np.clip(w * s, -15.5, 15.5).astype(F8E3)

    wiht = q8(_pack_kchunks_orig(np.ascontiguousarray(W['Wih'].T), 1536), sih)
    whht = _pack_kchunks_orig(np.ascontiguousarray(W['Whh'].T), 1536).astype(
        ml_dtypes.bfloat16)

    blobs = {
        'sA': sA,
        'big16': big16,
        'f32v': f32v,
        'whht': whht,
        'wc1': bf(_pack_kchunks_orig(W['Wc1'], 1024)),
        'gw16': gw16,
        'wiht0': np.ascontiguousarray(wiht[:, 0:4608]),
        'wiht1': np.ascontiguousarray(wiht[:, 4608:9216]),
        'wiht2': np.ascontiguousarray(wiht[:, 9216:13824]),
        'wiht3': np.ascontiguousarray(wiht[:, 13824:18432]),
    }
    return blobs, L, sih, shh


def _emit_fast(ctx, tc, d, out_dram, L, sih, shh):
    import concourse.mybir as mybir
    nc = tc.nc
    f32 = mybir.dt.float32
    f16 = mybir.dt.float16
    AF = mybir.ActivationFunctionType
    AL = mybir.AluOpType

    wts = ctx.enter_context(tc.tile_pool(name="wts", bufs=1))
    act = ctx.enter_context(tc.tile_pool(name="act", bufs=1))
    tmp = ctx.enter_context(tc.tile_pool(name="tmp", bufs=2))
    pu = ctx.enter_context(tc.tile_pool(name="pu", bufs=2, space="PSUM"))
    pvt = ctx.enter_context(tc.tile_pool(name="pvt", bufs=2, space="PSUM"))
    psm = ctx.enter_context(tc.tile_pool(name="psm", bufs=3, space="PSUM"))
    pgi = ctx.enter_context(tc.tile_pool(name="pgi", bufs=1, space="PSUM"))

    # ---- inputs -> SBUF, ordered by first use across the DMA queues ----
    sA = wts.tile([32, 584], f16, tag='sA')
    nc.sync.dma_start(sA[:, :], d['sA'])
    x0T = sA[:, 0:72]
    w1a = sA[:, 72:584]
    big16 = wts.tile([128, 74 + _SMCOLS], f16, tag='big16')
    nc.gpsimd.dma_start(big16[:, :], d['big16'])
    m72 = big16[0:72, 0:72]
    aggmask = big16[0:72, 0:NSPAN]   # [Mp.T | admask | dismask]
    masks = big16[0:72, 72:74]
    f32v = wts.tile([128, 29], f32, tag='f32v')
    nc.sync.dma_start(f32v[:, :], d['f32v'])
    # wiht3/whht/wc1 are needed late (gi drain/GRU/classifier); their
    # tiles live in thr (bufs=1) behind dummy writers keyed on GIN
    # progress, so their DMAs don't steal early bandwidth from gw16/wiht
    thr = ctx.enter_context(tc.tile_pool(name="thr", bufs=1))
    wiht3_dummy = thr.tile([1, 1], f16, tag='wiht3s')
    whht_dummy = thr.tile([1, 1], f16, tag='whht')
    wc1_dummy = thr.tile([1, 1], f16, tag='wc1')
    wiht3_tile = None

    def S(name):
        r0, nr, c0, ncn = _SLOTS[name]
        return big16[r0:r0 + nr, 74 + c0:74 + c0 + ncn]

    w1b_hi = wts.tile([128, 1024], f16, tag='w1b_hi')   # k-chunks 2,3
    w1b_lo = wts.tile([128, 1024], f16, tag='w1b_lo')   # k-chunks 0,1
    wha_t = wts.tile([128, 2048], f16, tag='wha')
    whb_t = wts.tile([128, 2048], f16, tag='whb')
    wiht_t = [wts.tile([128, 3 * 1536], mybir.dt.float8e3, tag=f'wiht{q}',
                       name=f'wiht{q}') for q in range(3)]
    nc.gpsimd.dma_start(w1b_hi[:, :], d['gw16'][:, 1024:2048])
    nc.gpsimd.dma_start(w1b_lo[:, :], d['gw16'][:, 0:1024])
    nc.gpsimd.dma_start(wha_t[:, :], d['gw16'][:, 2048:4096])
    nc.gpsimd.dma_start(whb_t[:, :], d['gw16'][:, 4096:6144])
    nc.gpsimd.dma_start(wiht_t[0][:, :], d['wiht0'])
    nc.gpsimd.dma_start(wiht_t[1][:, :], d['wiht1'])
    nc.gpsimd.dma_start(wiht_t[2][:, :], d['wiht2'])

    def wb_slice(l, fi, fo):
        if l == 0:
            t = w1b_hi if fi >= 2 else w1b_lo
            base = H * (fi % 2) + 128 * fo
            return t[:, base:base + 128]
        return whb_t[:, H * fi + 128 * fo:H * fi + 128 * fo + 128]

    def wiht_chunk(kc, j):
        q, r = divmod(kc, 3)
        base = 1536 * r + 128 * j
        t = wiht3_tile if q == 3 else wiht_t[q]
        return t[:, base:base + 128]

    # prefetch the sqrt ACT table (first LN would otherwise stall ~1.3us)
    sc1 = act.tile([1, 1], f32, tag='sc1')
    nc.vector.memset(sc1[:, :], 1.0)
    sc2 = act.tile([1, 1], f32, tag='sc2')
    eps = act.tile([72, 1], f32, tag='eps')
    nc.vector.memset(eps[:, :], LN_EPS)
    nc.scalar.activation(sc2[:, :], sc1[:, :], AF.Sqrt)

    gi_ps = pgi.tile([128, 24], f32, tag='gi')

    # ---- GIN layers ----------------------------------------------------
    # Activations live feature-major between layers as hnT [128, 4*NSPAN];
    # cols [NSPAN*c, NSPAN*c+72) are nodes, cols +72..74 the pooled feats.
    gi_backlog = []
    hnT_tiles = []
    gi_poffs = []
    hT, hcols = x0T, 32
    for l in range(3):
        wa = w1a if l == 0 else wha_t
        nk = max(hcols // 128, 1)

        # u = (Mp @ h) @ Wa.  For l>=1, hnT already carries (Mp @ h).T from
        # the previous layer's agg-folded transpose, so u comes straight
        # from those chunks; for l=0, z = x0 @ W1a then u = Mp @ z.
        if l == 0:
            z_h = [pu.tile([72, H // 2], f32, tag='pu', name=f'z{q}')
                   for q in range(2)]
            for q in range(2):
                nc.tensor.matmul(z_h[q][:, :], x0T,
                                 w1a[:, q * (H // 2):(q + 1) * (H // 2)],
                                 start=True, stop=True)
            z_sb = tmp.tile([72, H], f16, tag='z_sb')
            nc.vector.tensor_copy(z_sb[:, 0:H // 2], z_h[0][:, :])
            nc.scalar.copy(z_sb[:, H // 2:], z_h[1][:, :])
            u_h = [pu.tile([72, H // 2], f32, tag='pu', name=f'u{l}{q}')
                   for q in range(2)]
            for q in range(2):
                nc.tensor.matmul(u_h[q][:, :], m72,
                                 z_sb[:, q * (H // 2):(q + 1) * (H // 2)],
                                 start=True, stop=True)
        else:
            u_h = [pu.tile([72, H // 2], f32, tag='pu', name=f'u{l}{q}')
                   for q in range(2)]
            for ci, c in enumerate((0, 2, 1, 3)):
                for q in range(2):
                    nc.tensor.matmul(
                        u_h[q][:, :], hT[c][:, 0:72],
                        wa[:, H * c + q * (H // 2):H * c + (q + 1) * (H // 2)],
                        start=(ci == 0), stop=(ci == 3))

        # interleave layer-1's gi matmuls into layer-3's LN gap (their
        # wiht quarters have landed by then; earlier they'd stall on DMA)
        if l == 2:
            for fn in gi_backlog[:4]:
                fn()
            gi_backlog = gi_backlog[4:]

        # LN stats
        bst = tmp.tile([72, 12], f32, tag='bst')
        nc.vector.bn_stats(bst[:, 0:6], u_h[0][:, :])
        nc.vector.bn_stats(bst[:, 6:12], u_h[1][:, :])
        mv = tmp.tile([72, 2], f32, tag='mv')
        nc.vector.bn_aggr(mv[:, :], bst[:, :])
        negm = tmp.tile([72, 1], f32, tag='negm')
        nc.gpsimd.tensor_scalar(negm[:, :], mv[:, 0:1], -1.0, None, AL.mult)
        std = tmp.tile([72, 1], f32, tag='std')
        nc.scalar.activation(std[:, :], mv[:, 1:2], AF.Sqrt,
                             bias=eps[:, 0:1])
        rstd = tmp.tile([72, 1], f32, tag='rstd')
        nc.vector.reciprocal(rstd[:, :], std[:, :])
        # per-node 1/sigma folds into the transpose rhs, which also carries
        # the NEXT layer's aggregation (Mp.T) and the pooling masks; the
        # last layer only needs the pooled columns
        ncols = NSPAN if l < 2 else 2
        poff = 72 if l < 2 else 0
        dgm = tmp.tile([72, ncols], f16, tag='dgm')
        nc.vector.tensor_scalar_mul(
            dgm[:, :], aggmask if l < 2 else masks, rstd[:, 0:1])

        # r' = relu(u - m) node-major (the 1/sigma rides on dgm; relu
        # commutes with the positive scale); ACT half finishes first, so
        # its chunks (2,3) transpose first
        r_lo = tmp.tile([72, H // 2], f16, tag='r_lo')
        r_hi = tmp.tile([72, H // 2], f16, tag='r_hi')
        nc.scalar.activation(r_hi[:, :], u_h[1][:, :], AF.Relu,
                             bias=negm[:, 0:1], scale=1.0)
        us0 = tmp.tile([72, H // 2], f32, tag='us0')
        nc.vector.tensor_scalar_sub(us0[:, :], u_h[0][:, :], mv[:, 0:1])
        nc.vector.tensor_scalar_max(r_lo[:, :], us0[:, :], 0.0)

        # rT chunks: r'_chunk.T @ (rstd-scaled [Mp.T | masks]) -- the next
        # layer's aggregation and the pooling ride the transpose for free
        rT = tmp.tile([128, 4 * ncols], f16, tag='rT')
        for i, c in enumerate((2, 3, 0, 1)):
            src_r = r_hi if c >= 2 else r_lo
            tp = psm.tile([128, ncols], f32, tag='psm')
            nc.tensor.matmul(tp[:, :],
                             src_r[:, 128 * (c % 2):128 * (c % 2) + 128],
                             dgm, start=True, stop=True)
            if i % 2 == 0:
                nc.vector.tensor_copy(rT[:, ncols * c:ncols * (c + 1)], tp[:, :])
            else:
                nc.scalar.copy(rT[:, ncols * c:ncols * (c + 1)], tp[:, :])

        # vT chunks = Wb-chunk.T @ rT-chunk
        vt_ps = [pvt.tile([128, 2 * ncols], f32, tag='pvt', name=f'vt{l}{q}')
                 for q in range(2)]
        FI = (2, 3, 0, 1)
        for ki, fi in enumerate(FI):
            for fo in range(4):
                q, o = fo % 2, fo // 2
                nc.tensor.matmul(
                    vt_ps[q][:, ncols * o:ncols * (o + 1)],
                    wb_slice(l, fi, fo),
                    rT[:, ncols * fi:ncols * (fi + 1)],
                    start=(ki == 0 and fo < 2), stop=(ki == 3),
                    skip_group_check=True)
        hnT = [act.tile([128, ncols], f16, tag=f'hnT{l}{fo}',
                        name=f'hnT{l}{fo}') for fo in range(4)]
        for fo in range(4):
            q, o = fo % 2, fo // 2
            srcp = vt_ps[q][:, ncols * o:ncols * (o + 1)]
            if fo < 2:
                nc.vector.tensor_copy(hnT[fo][:, :], srcp)
            else:
                nc.scalar.copy(hnT[fo][:, :], srcp)
        hnT_tiles.append(hnT)
        gi_poffs.append(poff)

        # queue this layer's gi matmuls; rhs = the pooled cols of hnT
        def make_gi(lv, kcv, mcv):
            def emit_gi():
                src = hnT_tiles[lv][mcv]
                po = gi_poffs[lv]
                for j in range(12):
                    nc.tensor.matmul(
                        gi_ps[:, 2 * j:2 * j + 2],
                        wiht_chunk(kcv, j),
                        src[:, po:po + 2],
                        start=(kcv == 0 and j == 0), stop=(kcv == 11),
                        skip_group_check=True)
            return emit_gi
        for mc in range(4):
            gi_backlog.append(make_gi(l, 4 * l + mc, mc))
        if l == 0:
            nc.vector.tensor_copy(wiht3_dummy[0:1, 0:1], hnT[0][0:1, 0:1])
            wiht3_tile = thr.tile([128, 3 * 1536], mybir.dt.float8e3,
                                  tag='wiht3s', name='wiht3s')
            nc.sync.dma_start(wiht3_tile[:, :], d['wiht3'])
            nc.vector.tensor_copy(whht_dummy[0:1, 0:1], hnT[0][0:1, 0:1])
            whht = thr.tile([128, 4 * 1536], mybir.dt.bfloat16, tag='whht')
            nc.sync.dma_start(whht[:, :], d['whht'])
        elif l == 2:
            nc.vector.tensor_copy(wc1_dummy[0:1, 0:1], hnT[0][0:1, 0:1])
            wc1 = thr.tile([128, 4 * 1024], f16, tag='wc1')
            nc.sync.dma_start(wc1[:, :], d['wc1'])
        hT = hnT
        hcols = H

    for fn in gi_backlog:
        fn()

    # switch ACT to the sigmoid/tanh table while PE drains gi matmuls;
    # reading layer-3 output pins this after the last LN sqrt
    nc.scalar.activation(sc2[:, :], hnT_tiles[2][0][0:1, 0:1], AF.Sigmoid)

    # ---- GRU setup + step 0 (h=0 so gr=0; gates come from gib2) ----
    gib2 = act.tile([128, 24], f32, tag='gib2')
    nc.vector.scalar_tensor_tensor(gib2[:, :], gi_ps[:, :], 1.0 / sih,
                                   f32v[:, 0:24], AL.mult, AL.add)
    g0 = 0 if L > 1 else 1
    z0 = tmp.tile([128, 4], f32, tag='z0')
    nc.scalar.activation(z0[:, :], gib2[:, 8 + g0:16:2], AF.Sigmoid)
    n = tmp.tile([128, 4], f32, tag='n')
    nc.scalar.activation(n[:, :], gib2[:, 16 + g0::2], AF.Tanh)
    w = tmp.tile([128, 4], f32, tag='w')
    nc.vector.tensor_scalar(w[:, :], z0[:, :], -1.0, 1.0, AL.mult, AL.add)
    # h state as two half-tiles so the next step's first matmuls start as
    # soon as the first half lands
    h01 = tmp.tile([128, 2], f16, tag='h01')
    nc.vector.tensor_tensor(h01[:, :], w[:, 0:2], n[:, 0:2], AL.mult)
    h23 = tmp.tile([128, 2], f16, tag='h23')
    nc.vector.tensor_tensor(h23[:, :], w[:, 2:4], n[:, 2:4], AL.mult)

    # r/z bias closers for steps >= 1 (only needed after step 0's gates)
    gibT = []  # per graph: [8, 128] fp16 (r rows 0:4, z rows 4:8)
    for g in range(2):
        gb = tmp.tile([128, 8], f16, tag='gib_h')
        nc.vector.tensor_copy(gb[:, :], gib2[:, g:16:2])
        tp = psm.tile([8, 128], f16, tag='psm')
        nc.tensor.transpose(tp[:, :], gb[:, :], S('eye128'))
        t = act.tile([8, 128], f16, tag=f'gibT{g}')
        nc.scalar.activation(t[:, :], tp[:, :], AF.Identity, scale=shh)
        gibT.append(t)

    def hcol(c):
        return (h01 if c < 2 else h23)[:, c % 2:c % 2 + 1]

    eye4 = S('eye8')[0:4, 0:4]
    for t in range(1, L):
        gs = 0 if t < L - 1 else 1
        rz_ps = psm.tile([128, 8], f32, tag='psm')
        grn = psm.tile([128, 4], f32, tag='psm')
        for j in range(8):
            for c in range(4):
                nc.tensor.matmul(
                    rz_ps[:, j:j + 1],
                    whht[:, 1536 * c + 128 * j:1536 * c + 128 * (j + 1)],
                    hcol(c), start=(c == 0 and j == 0),
                    stop=False, skip_group_check=True)
        nc.tensor.matmul(rz_ps[:, :], gibT[gs][:, :], S('eye8'),
                         start=False, stop=True, skip_group_check=True)
        for j in range(8, 12):
            for c in range(4):
                nc.tensor.matmul(
                    grn[:, j - 8:j - 7],
                    whht[:, 1536 * c + 128 * j:1536 * c + 128 * (j + 1)],
                    hcol(c), start=(c == 0 and j == 8),
                    stop=(c == 3 and j == 11), skip_group_check=True)

        rzs = tmp.tile([128, 8], f32, tag='rzs')
        nc.scalar.activation(rzs[:, :], rz_ps[:, :], AF.Sigmoid,
                             scale=1.0 / shh)
        zh = tmp.tile([128, 4], f32, tag='zh')
        nc.vector.tensor_tensor(zh[:, 0:2], rzs[:, 4:6], h01[:, :], AL.mult)
        nc.vector.tensor_tensor(zh[:, 2:4], rzs[:, 6:8], h23[:, :], AL.mult)
        w = tmp.tile([128, 4], f32, tag='w')
        nc.gpsimd.tensor_scalar(w[:, :], rzs[:, 4:8], -1.0, 1.0,
                                AL.mult, AL.add)
        nt = tmp.tile([128, 4], f32, tag='nt')
        nc.vector.scalar_tensor_tensor(nt[:, :], grn[:, :], 1.0 / shh,
                                       rzs[:, 0:4], AL.mult, AL.mult)
        nc.vector.tensor_tensor(nt[:, :], nt[:, :], gib2[:, 16 + gs::2],
                                AL.add)
        n = tmp.tile([128, 4], f32, tag='n')
        nc.scalar.activation(n[:, :], nt[:, :], AF.Tanh)
        wn = tmp.tile([128, 4], f32, tag='wn')
        nc.vector.tensor_tensor(wn[:, :], w[:, :], n[:, :], AL.mult)
        h01 = tmp.tile([128, 2], f16, tag='h01')
        nc.vector.tensor_tensor(h01[:, :], wn[:, 0:2], zh[:, 0:2], AL.add)
        h23 = tmp.tile([128, 2], f16, tag='h23')
        nc.gpsimd.tensor_tensor(h23[:, :], wn[:, 2:4], zh[:, 2:4], AL.add)

    # ---- classifier ----
    hid_ps = psm.tile([128, 8], f32, tag='psm')
    for mc in range(8):
        for c in range(4):
            nc.tensor.matmul(
                hid_ps[:, mc:mc + 1],
                wc1[:, 1024 * c + 128 * mc:1024 * c + 128 * (mc + 1)],
                hcol(c), start=(c == 0 and mc == 0),
                stop=(c == 3 and mc == 7), skip_group_check=True)
    hid = tmp.tile([128, 8], f16, tag='hid_sb')
    nc.scalar.activation(hid[:, :], hid_ps[:, :], AF.Relu)
    fin_ps = psm.tile([1, 1], f32, tag='psm')
    for mc in range(8):
        nc.tensor.matmul(fin_ps[:, :], hid[:, mc:mc + 1], S('wc2')[:, mc:mc + 1],
                         start=(mc == 0), stop=(mc == 7))
    out_sb = tmp.tile([1, 1], f32, tag='out_sb')
    nc.scalar.activation(out_sb[:, :], fin_ps[:, :], AF.Identity,
                         bias=f32v[0:1, 28:29], scale=1.0)
    nc.sync.dma_start(out_dram, out_sb[:, :], single_packet=True)


# ------------------------------------------------------------ general path
# (previous fully-general emission; used when biases/LN affine are nonzero)

_XSLOTS = {}
_XCOLS = 0


def _xslot(name, nrows, ncols):
    global _XCOLS
    _XSLOTS[name] = (nrows, _XCOLS, ncols)
    _XCOLS += ncols


_xslot('x0', 72, 32)
_xslot('eye72', 72, 72)
_xslot('w1a', 32, H)
_xslot('m72', 72, 72)
_xslot('ones72', 1, 72)
_xslot('brows', 1, 4 * H + 2)

_GSLOTS = {}
_GSMCOLS = 0


def _gslot(name, nrows, ncols):
    global _GSMCOLS
    _GSLOTS[name] = (0, nrows, _GSMCOLS, ncols)
    _GSMCOLS += ncols


_gslot('bc1t', 8, 128)
_gslot('bhhnt', 4, 128)
_gslot('eye8', 8, 8)
_gslot('wc2', 128, 8)
_gslot('eye128', 128, 128)


def _prep_general(inputs):
    f32 = np.float32

    def bf(x):
        return np.asarray(x, f32).astype(F16)

    x = np.asarray(inputs['x_embedded'], f32)
    tei = np.asarray(inputs['template_edge_index']).astype(np.int64)
    L = int(np.asarray(inputs['LOS_batch']).reshape(-1)[0])

    A = np.zeros((36, 36), f32)
    np.add.at(A, (tei[1], tei[0]), 1.0)
    Mp = A + np.eye(36, dtype=f32)
    m72 = np.zeros((72, 72), f32)
    m72[:36, :36] = Mp.T
    m72[36:, 36:] = Mp.T

    W = {k: np.asarray(v, f32) for k, v in inputs.items()
         if k not in ('x_embedded', 'template_edge_index', 'LOS_batch')}

    xvals = {
        'x0': x,
        'eye72': np.eye(72, dtype=f32),
        'w1a': W['W1a'],
        'm72': m72,
        'ones72': np.ones((1, 72), f32),
        'brows': np.concatenate(
            [W['b1a'], W['b1b'], W['bha'], W['bhb'], [0.0], [0.0]]
        ).reshape(1, 4 * H + 2),
    }
    xe16 = np.zeros((72, _XCOLS), F16)
    for name, (nr, c0, ncn) in _XSLOTS.items():
        xe16[0:nr, c0:c0 + ncn] = bf(xvals[name])

    vals = {
        'bc1t': W['bc1'].reshape(8, 128),
        'bhhnt': W['bhh'][2 * H:].reshape(4, 128),
        'eye8': np.eye(8, dtype=f32),
        'wc2': np.ascontiguousarray(W['Wc2'].reshape(8, 128).T),
        'eye128': np.eye(128, dtype=f32),
    }
    sm16 = np.zeros((128, _GSMCOLS), F16)
    for name, (r0, nr, c0, ncn) in _GSLOTS.items():
        sm16[r0:r0 + nr, c0:c0 + ncn] = bf(vals[name])

    f32v = np.zeros((128, 53), f32)
    bih_t = W['bih'].reshape(12, 128).T
    bhh_t = W['bhh'].reshape(12, 128).T
    combo = bih_t.copy()
    combo[:, 0:8] += bhh_t[:, 0:8]
    f32v[:, 0:24:2] = combo
    f32v[:, 1:24:2] = combo
    f32v[:, 24:28] = bhh_t[:, 8:12]
    f32v[:, 28] = W['bc2'][0]
    f32v[:, 29:33] = W['g1'].reshape(4, 128).T
    f32v[:, 33:37] = W['be1'].reshape(4, 128).T
    f32v[:, 37:41] = W['gh'].reshape(4, 128).T
    f32v[:, 41:45] = W['beh'].reshape(4, 128).T
    f32v[:, 45:49] = W['b1b'].reshape(4, 128).T
    f32v[:, 49:53] = W['bhb'].reshape(4, 128).T

    gw16 = np.concatenate([
        _pack_kchunks_orig(W['W1b'], H), _pack_kchunks_orig(W['Wha'], H),
        _pack_kchunks_orig(W['Whb'], H)], axis=1).astype(F16)

    blobs = {
        'xe0': xe16[:, 0:104].copy(),
        'xe16': xe16,
        'sm16': sm16,
        'gw16': gw16,
        'f32v': f32v,
        'wiht': bf(_pack_kchunks_orig(np.ascontiguousarray(W['Wih'].T), 1536)),
        'whht': bf(_pack_kchunks_orig(np.ascontiguousarray(W['Whh'].T), 1536)),
        'wc1': bf(_pack_kchunks_orig(W['Wc1'], 1024)),
    }
    return blobs, L


def _emit_general(ctx, tc, d, out_dram, L):
    import concourse.mybir as mybir
    nc = tc.nc
    f32 = mybir.dt.float32
    f16 = mybir.dt.float16
    AF = mybir.ActivationFunctionType
    AL = mybir.AluOpType

    wts = ctx.enter_context(tc.tile_pool(name="wts", bufs=1))
    act = ctx.enter_context(tc.tile_pool(name="act", bufs=1))
    tmp = ctx.enter_context(tc.tile_pool(name="tmp", bufs=2))
    pu = ctx.enter_context(tc.tile_pool(name="pu", bufs=2, space="PSUM"))
    pvt = ctx.enter_context(tc.tile_pool(name="pvt", bufs=2, space="PSUM"))
    psm = ctx.enter_context(tc.tile_pool(name="psm", bufs=3, space="PSUM"))
    pgi = ctx.enter_context(tc.tile_pool(name="pgi", bufs=1, space="PSUM"))

    xe0 = wts.tile([72, 104], f16, tag='xe0')
    nc.sync.dma_start(xe0[:, :], d['xe0'])
    x0s = xe0[:, 0:32]
    eye72 = xe0[:, 32:104]
    xe16 = wts.tile([72, _XCOLS], f16, tag='xe16')
    nc.sync.dma_start(xe16[:, :], d['xe16'])

    def X(name):
        nr, c0, ncn = _XSLOTS[name]
        return xe16[0:nr, c0:c0 + ncn]

    sm16 = wts.tile([128, _GSMCOLS], f16, tag='sm16')
    nc.sync.dma_start(sm16[:, :], d['sm16'])
    f32v = wts.tile([128, 53], f32, tag='f32v')
    nc.sync.dma_start(f32v[:, :], d['f32v'])
    whht = wts.tile([128, 4 * 1536], f16, tag='whht')
    nc.sync.dma_start(whht[:, :], d['whht'])
    wc1 = wts.tile([128, 4 * 1024], f16, tag='wc1')
    nc.sync.dma_start(wc1[:, :], d['wc1'])

    def S(name):
        r0, nr, c0, ncn = _GSLOTS[name]
        return sm16[r0:r0 + nr, c0:c0 + ncn]

    gw16 = wts.tile([128, 3 * 4 * H], f16, tag='gw16')
    wiht_t = [wts.tile([128, 3 * 1536], f16, tag=f'wiht{q}',
                       name=f'wiht{q}') for q in range(4)]
    nc.gpsimd.dma_start(gw16[:, 0:2048], d['gw16'][:, 0:2048])
    nc.gpsimd.dma_start(wiht_t[0][:, :], d['wiht'][:, 0:4608])
    nc.gpsimd.dma_start(gw16[:, 2048:4096], d['gw16'][:, 2048:4096])
    nc.gpsimd.dma_start(wiht_t[1][:, :], d['wiht'][:, 4608:9216])
    nc.gpsimd.dma_start(gw16[:, 4096:6144], d['gw16'][:, 4096:6144])
    nc.gpsimd.dma_start(wiht_t[2][:, :], d['wiht'][:, 9216:13824])
    nc.gpsimd.dma_start(wiht_t[3][:, :], d['wiht'][:, 13824:18432])

    def wiht_chunk(kc, j):
        q, r = divmod(kc, 3)
        base = 1536 * r + 128 * j
        return wiht_t[q][:, base:base + 128]

    sc1 = act.tile([1, 1], f32, tag='sc1')
    nc.vector.memset(sc1[:, :], 1.0)
    sc2 = act.tile([1, 1], f32, tag='sc2')
    eps = act.tile([72, 1], f32, tag='eps')
    nc.vector.memset(eps[:, :], LN_EPS)
    nc.scalar.activation(sc2[:, :], sc1[:, :], AF.Sqrt)

    featsT = act.tile([128, 24], f16, tag='featsT')
    gi_ps = pgi.tile([128, 24], f32, tag='gi')

    x0T = tmp.tile([32, 72], f16, tag='x0T')
    tp0 = psm.tile([128, 72], f16, tag='psm')
    nc.tensor.transpose(tp0[0:32, :], x0s, eye72)
    nc.vector.tensor_copy(x0T[:, :], tp0[0:32, :])

    gi_backlog = []
    hT = x0T
    hcols = 32
    for l in range(3):
        wa = X('w1a') if l == 0 else gw16[:, 2048:4096]
        wb = gw16[:, 0:2048] if l == 0 else gw16[:, 4096:6144]
        ba_off = 0 if l == 0 else 2 * H
        gcol = 29 if l == 0 else 37
        becol = 33 if l == 0 else 41
        bbtcol = 45 if l == 0 else 49
        nk = max(hcols // 128, 1)

        z_h = [pbig.tile([72, H // 2], f32, tag='pbig', name=f'z{q}')
               for q in range(2)]
        for c in range(nk):
            cs = min(128, hcols - 128 * c)
            for q in range(2):
                rhs = (wa if l == 0 else wa[:, H * c:H * (c + 1)])[
                    :, q * (H // 2):(q + 1) * (H // 2)]
                nc.tensor.matmul(z_h[q][:, :],
                                 hT[0:cs, 72 * c:72 * (c + 1)], rhs,
                                 start=(c == 0), stop=(c == nk - 1))
        z_sb = tmp.tile([72, H], f16, tag='z_sb')
        nc.vector.tensor_copy(z_sb[:, 0:H // 2], z_h[0][:, :])
        nc.scalar.copy(z_sb[:, H // 2:], z_h[1][:, :])

        u_h = [pbig.tile([72, H // 2], f32, tag='pbig', name=f'u{q}')
               for q in range(2)]
        for q in range(2):
            nc.tensor.matmul(u_h[q][:, :], X('m72'),
                             z_sb[:, q * (H // 2):(q + 1) * (H // 2)],
                             start=True, stop=False)
        for q in range(2):
            off = ba_off + q * (H // 2)
            nc.tensor.matmul(u_h[q][:, :], X('ones72'),
                             X('brows')[:, off:off + H // 2],
                             start=False, stop=True)

        bst = tmp.tile([72, 12], f32, tag='bst')
        nc.vector.bn_stats(bst[:, 0:6], u_h[0][:, :])
        nc.vector.bn_stats(bst[:, 6:12], u_h[1][:, :])
        mv = tmp.tile([72, 2], f32, tag='mv')
        nc.vector.bn_aggr(mv[:, :], bst[:, :])
        std = tmp.tile([72, 1], f32, tag='std')
        nc.scalar.activation(std[:, :], mv[:, 1:2], AF.Sqrt,
                             bias=eps[:, 0:1])
        rstd = tmp.tile([72, 1], f32, tag='rstd')
        nc.vector.reciprocal(rstd[:, :], std[:, :])
        mb = tmp.tile([72, 1], f32, tag='mb')
        nc.vector.scalar_tensor_tensor(mb[:, :], mv[:, 0:1], -1.0,
                                       rstd[:, 0:1], AL.mult, AL.mult)

        us = tmp.tile([72, H], f16, tag='us')
        nc.vector.tensor_scalar(us[:, 0:H // 2], u_h[0][:, :],
                                mv[:, 0:1], rstd[:, 0:1],
                                AL.subtract, AL.mult)
        nc.scalar.activation(us[:, H // 2:], u_h[1][:, :], AF.Identity,
                             bias=mb[:, 0:1], scale=rstd[:, 0:1])
        rT = tmp.tile([128, 4 * 72], f16, tag='rT')
        for c in range(4):
            tp = psm.tile([128, 72], f16, tag='psm')
            nc.tensor.transpose(tp[:, :], us[:, 128 * c:128 * (c + 1)],
                                eye72)
            nc.scalar.activation(rT[:, 72 * c:72 * (c + 1)], tp[:, :], AF.Relu,
                                 bias=f32v[:, becol + c:becol + c + 1],
                                 scale=f32v[:, gcol + c:gcol + c + 1])

        vt_ps = [pbig.tile([128, 2 * 72], f32, tag='pvt', name=f'vt{q}')
                 for q in range(2)]
        for fi in range(4):
            for fo in range(4):
                q, o = fo % 2, fo // 2
                nc.tensor.matmul(
                    vt_ps[q][:, 72 * o:72 * (o + 1)],
                    wb[:, H * fi + 128 * fo:H * fi + 128 * fo + 128],
                    rT[:, 72 * fi:72 * (fi + 1)],
                    start=(fi == 0 and fo < 2), stop=(fi == 3),
                    skip_group_check=True)
        hnT = tmp.tile([128, 4 * 72], f16, tag='hnT')
        for fo in range(4):
            q, o = fo % 2, fo // 2
            dst = hnT[:, 72 * fo:72 * (fo + 1)]
            srcp = vt_ps[q][:, 72 * o:72 * (o + 1)]
            bb = f32v[:, bbtcol + fo:bbtcol + fo + 1]
            if fo < 2:
                nc.vector.tensor_scalar_add(dst, srcp, bb[:, 0:1])
            else:
                nc.scalar.activation(dst, srcp, AF.Identity, bias=bb[:, 0:1])

        pf = tmp.tile([128, 8], f32, tag='pf')
        for fo in range(4):
            for g in range(2):
                nc.vector.tensor_reduce(
                    pf[:, 2 * fo + g:2 * fo + g + 1],
                    hnT[:, 72 * fo + 36 * g:72 * fo + 36 * g + 36],
                    mybir.AxisListType.X, AL.add)
        nc.vector.tensor_copy(featsT[:, 8 * l:8 * l + 8], pf[:, :])

        def make_gi(kcv):
            def emit_gi():
                for j in range(12):
                    nc.tensor.matmul(
                        gi_ps[:, 2 * j:2 * j + 2],
                        wiht_chunk(kcv, j),
                        featsT[:, 2 * kcv:2 * kcv + 2],
                        start=(kcv == 0 and j == 0), stop=(kcv == 11),
                        skip_group_check=True)
            return emit_gi
        for mc in range(4):
            gi_backlog.append(make_gi(4 * l + mc))
        hT = hnT
        hcols = H

    for kc in range(12):
        gi_backlog[kc]()
    gi_backlog = []

    gib2 = act.tile([128, 24], f32, tag='gib2')
    nc.vector.tensor_tensor(gib2[:, :], gi_ps[:, :], f32v[:, 0:24], AL.add)
    gibT = []
    for g in range(2):
        pair = []
        for half in range(2):
            gb = tmp.tile([128, 4], f16, tag='gib_h')
            nc.vector.tensor_copy(gb[:, :], gib2[:, g + 8 * half:g + 8 * half + 8:2])
            tp = psm.tile([4, 128], f16, tag='psm')
            nc.tensor.transpose(tp[:, :], gb[:, :], S('eye128'))
            t = act.tile([4, 128], f16, tag=f'gibT{g}{half}')
            nc.vector.tensor_copy(t[:, :], tp[:, :])
            pair.append(t)
        gibT.append(pair)

    g0 = 0 if L > 1 else 1
    rz = tmp.tile([128, 8], f32, tag='rz')
    nc.scalar.activation(rz[:, :], gib2[:, g0:16:2], AF.Sigmoid)
    nt = tmp.tile([128, 4], f32, tag='nt')
    nc.vector.tensor_tensor(nt[:, :], rz[:, 0:4], f32v[:, 24:28], AL.mult)
    nc.vector.tensor_tensor(nt[:, :], nt[:, :], gib2[:, 16 + g0::2], AL.add)
    n = tmp.tile([128, 4], f32, tag='n')
    nc.scalar.activation(n[:, :], nt[:, :], AF.Tanh)
    w = tmp.tile([128, 4], f32, tag='w')
    nc.gpsimd.tensor_scalar(w[:, :], rz[:, 4:8], -1.0, 1.0, AL.mult, AL.add)
    h_f = tmp.tile([128, 4], f32, tag='h_f')
    nc.gpsimd.tensor_tensor(h_f[:, :], w[:, :], n[:, :], AL.mult)
    h_b = tmp.tile([128, 4], f16, tag='h_b')
    nc.vector.tensor_tensor(h_b[:, :], w[:, :], n[:, :], AL.mult)

    eye4 = S('eye8')[0:4, 0:4]
    for t in range(1, L):
        gs = 0 if t < L - 1 else 1
        grr = psm.tile([128, 4], f32, tag='psm')
        grn = psm.tile([128, 4], f32, tag='psm')
        grz = psm.tile([128, 4], f32, tag='psm')
        for out_ps, js, closer in (
                (grr, range(0, 4), (gibT[gs][0][:, :], eye4)),
                (grn, range(8, 12), (S('bhhnt'), eye4)),
                (grz, range(4, 8), (gibT[gs][1][:, :], eye4))):
            j0 = js[0]
            for j in js:
                for c in range(4):
                    nc.tensor.matmul(
                        out_ps[:, j - j0:j - j0 + 1],
                        whht[:, 1536 * c + 128 * j:1536 * c + 128 * (j + 1)],
                        h_b[:, c:c + 1], start=(c == 0 and j == j0),
                        stop=False, skip_group_check=True)
            nc.tensor.matmul(out_ps[:, :], closer[0], closer[1],
                             start=False, stop=True, skip_group_check=True)

        r = tmp.tile([128, 4], f32, tag='r')
        nc.scalar.activation(r[:, :], grr[:, :], AF.Sigmoid)
        nt = tmp.tile([128, 4], f32, tag='nt')
        nc.vector.tensor_tensor(nt[:, :], r[:, :], grn[:, :], AL.mult)
        nc.vector.tensor_tensor(nt[:, :], nt[:, :], gib2[:, 16 + gs::2],
                                AL.add)
        n = tmp.tile([128, 4], f32, tag='n')
        nc.scalar.activation(n[:, :], nt[:, :], AF.Tanh)
        z = tmp.tile([128, 4], f32, tag='z')
        nc.scalar.activation(z[:, :], grz[:, :], AF.Sigmoid)
        zh = tmp.tile([128, 4], f32, tag='zh')
        nc.gpsimd.tensor_tensor(zh[:, :], z[:, :], h_f[:, :], AL.mult)
        w = tmp.tile([128, 4], f32, tag='w')
        nc.gpsimd.tensor_scalar(w[:, :], z[:, :], -1.0, 1.0, AL.mult, AL.add)
        wn = tmp.tile([128, 4], f32, tag='wn')
        nc.vector.tensor_tensor(wn[:, :], w[:, :], n[:, :], AL.mult)
        h_f = tmp.tile([128, 4], f32, tag='h_f')
        nc.gpsimd.tensor_tensor(h_f[:, :], wn[:, :], zh[:, :], AL.add)
        h_b = tmp.tile([128, 4], f16, tag='h_b')
        nc.vector.tensor_tensor(h_b[:, :], wn[:, :], zh[:, :], AL.add)

    hid_ps = psm.tile([128, 8], f32, tag='psm')
    for mc in range(8):
        for c in range(4):
            nc.tensor.matmul(
                hid_ps[:, mc:mc + 1],
                wc1[:, 1024 * c + 128 * mc:1024 * c + 128 * (mc + 1)],
                h_b[:, c:c + 1], start=(c == 0 and mc == 0), stop=False,
                skip_group_check=True)
    nc.tensor.matmul(hid_ps[:, :], S('bc1t'), S('eye8'),
                     start=False, stop=True, skip_group_check=True)
    hid = tmp.tile([128, 8], f16, tag='hid_sb')
    nc.scalar.activation(hid[:, :], hid_ps[:, :], AF.Relu)
    fin_ps = psm.tile([1, 1], f32, tag='psm')
    for mc in range(8):
        nc.tensor.matmul(fin_ps[:, :], hid[:, mc:mc + 1], S('wc2')[:, mc:mc + 1],
                         start=(mc == 0), stop=(mc == 7))
    out_sb = tmp.tile([1, 1], f32, tag='out_sb')
    nc.scalar.activation(out_sb[:, :], fin_ps[:, :], AF.Identity,
                         bias=f32v[0:1, 28:29], scale=1.0)
    nc.sync.dma_start(out_dram, out_sb[:, :])


# --------------------------------------------------------------- plumbing

def _is_fast_path(inputs):
    f32 = np.float32
    zeros = ['b1a', 'be1', 'b1b', 'bha', 'beh', 'bhb', 'bhh', 'bc1']
    ones = ['g1', 'gh']
    for k in zeros:
        if np.any(np.asarray(inputs[k], f32) != 0.0):
            return False
    for k in ones:
        if np.any(np.asarray(inputs[k], f32) != 1.0):
            return False
    return True


def _build_program(key, blobs, emit_fn):
    from contextlib import ExitStack
    import concourse.bacc as bacc
    import concourse.tile as tile
    import concourse.mybir as mybir

    nc = bacc.Bacc("TRN2", target_bir_lowering=False, debug=False,
                   num_devices=8)
    d = {}
    for name, arr in blobs.items():
        d[name] = nc.dram_tensor(name, list(arr.shape),
                                 mybir.dt.from_np(arr.dtype),
                                 kind="ExternalInput").ap()
    out_dram = nc.dram_tensor("out", [1], mybir.dt.float32,
                              kind="ExternalOutput").ap()
    with tile.TileContext(nc) as tc:
        with ExitStack() as ctx:
            emit_fn(ctx, tc, d, out_dram)
    nc.compile()
    return nc


def _install_ntff_hook():
    """The agent image's antenv lacks axon_hooks; recreate it so
    run_bass_kernel_spmd(trace=True) can capture NTFF profiles."""
    import sys, types
    try:
        import antenv
        if 'antenv.axon_hooks' in sys.modules:
            return
        mod = types.ModuleType('antenv.axon_hooks')
        mod._hook = None

        def set_axon_ntff_profile_hook(hk):
            mod._hook = hk

        def get_axon_ntff_profile_hook():
            return mod._hook

        mod.set_axon_ntff_profile_hook = set_axon_ntff_profile_hook
        mod.get_axon_ntff_profile_hook = get_axon_ntff_profile_hook
        sys.modules['antenv.axon_hooks'] = mod
        antenv.axon_hooks = mod
        from trn_agent_boot.trn_boot import _ntff_profile_via_ctypes
        so = '/opt/axon/libaxon_pjrt.so'
        if os.path.exists(so):
            mod._hook = _ntff_profile_via_ctypes(so)
    except Exception as e:  # profiling is best-effort
        print(f"ntff hook install failed: {e}")


def kernel(**inputs):
    from concourse.bass_utils import run_bass_kernel_spmd

    fast = _is_fast_path(inputs)
    if fast:
        blobs, L, sih, shh = _prep_fast(inputs)
        key = ('fast', L)
        if key not in _prog_cache:
            _prog_cache[key] = _build_program(
                key, blobs,
                lambda ctx, tc, d, o: _emit_fast(ctx, tc, d, o, L, sih, shh))
    else:
        blobs, L = _prep_general(inputs)
        key = ('gen', L)
        if key not in _prog_cache:
            _prog_cache[key] = _build_program(
                key, blobs,
                lambda ctx, tc, d, o: _emit_general(ctx, tc, d, o, L))
    nc = _prog_cache[key]

    in_maps = [dict(blobs) for _ in range(8)]
    trace = bool(int(os.environ.get('KERNEL_TRACE', '0')))
    if trace:
        _install_ntff_hook()
    res = run_bass_kernel_spmd(nc, in_maps, list(range(8)), trace=trace)
    last_run_info['exec_time_ns'] = res.exec_time_ns
    last_run_info['results'] = res
    return np.asarray(res.results[0]['out'], np.float32).reshape(1)


# revision 17
# speedup vs baseline: 1.0231x; 1.0077x over previous
"""Trainium2 Bass kernel for nn_ExplainerCompatibleGinGru.

Math: the reference pads the batch with 31 zero graphs, splits the node dim
into two 36-node graphs (ad = rows 0:36, dis = rows 36:72), runs 3 GIN layers
with sum-pooling, packs [ad x (L-1), dis] as a GRU sequence per batch
element, and returns out[0] -- which depends ONLY on graph 0 (ad), graph 32
(dis) and L = LOS_batch[0].  So the kernel computes: GIN on the stacked
72-node 2-graph block, an L-step GRU on one sequence, and a tiny classifier.

Runs replicated on all 8 cores (one latency-bound dependency chain;
collectives have a ~5us floor, so no sharding).

Fast path (taken when all GIN/classifier biases are zero and LN gain/bias
are 1/0, as produced by setup_inputs):
- Wih and Whh are stored as pow2-scaled float8e3 (e3m4): LDWEIGHTS streams
  4 B/cycle with FWL vs 2 for fp16 -- these two matrices are ~75% of all
  PE weight-load traffic.  Descales fold into existing ACT scale / DVE
  scalar_tensor_tensor slots, so no extra instructions.
- LN apply + relu fuse into the PSUM evacuation (ACT relu(bias=-m*rstd,
  scale=rstd) / DVE 2-op), so the transpose input is already relu'd.
- The per-layer transpose is a general matmul against [eye72 | graph-masks]:
  the two extra columns produce the sum-pooled features for free inside the
  Wb matmuls' accumulation -- no tensor_reduce pooling pass at all.  The gi
  (Wih) matmuls read their rhs straight out of each layer's hnT tile.
- GRU r+z gates share one PSUM tile -> one bias closer + one sigmoid.
- Single fp16 h state (no parallel f32 copy); shorter gate tail.
- Sqrt and Sigmoid/Tanh live in different ACT table sets: the sigmoid set
  is preloaded via a dummy op while the PE drains gi matmuls, off the
  critical path.

The general path (nonzero biases / LN affine) falls back to the previous
fully-general emission.
"""

import os
import numpy as np
import ml_dtypes

F16 = np.float16
F8E3 = ml_dtypes.float8_e3m4

H = 512
LN_EPS = 1e-5
NSPAN = 74   # 72 node cols + 2 pooled cols per feature chunk

_prog_cache = {}
last_run_info = {}


def _pow2_scale(w, target=12.0):
    am = float(np.abs(w).max())
    if am == 0.0:
        return 1.0
    return float(2.0 ** np.floor(np.log2(target / am)))


def _pack_kchunks_orig(w, ncols):
    """[K, N] weight -> [128, (K//128)*N], chunk kc at cols [N*kc, N*(kc+1))."""
    k, n = w.shape
    assert k % 128 == 0 and n == ncols
    nk = k // 128
    return np.ascontiguousarray(
        w.reshape(nk, 128, n).transpose(1, 0, 2).reshape(128, nk * n))


# ---------------------------------------------------------------- fast path

# sm16 blob layout: name -> (row0, nrows, col0, ncols)
_SLOTS = {}
_SMCOLS = 0


def _slot(name, nrows, ncols):
    global _SMCOLS
    _SLOTS[name] = (0, nrows, _SMCOLS, ncols)
    _SMCOLS += ncols


_slot('bc1t', 8, 128)
_slot('bhhnt', 4, 128)
_slot('eye8', 8, 8)
_slot('wc2', 128, 8)
_slot('eye128', 128, 128)


def _prep_fast(inputs):
    f32 = np.float32

    def bf(x):
        return np.asarray(x, f32).astype(F16)

    x = np.asarray(inputs['x_embedded'], f32)
    tei = np.asarray(inputs['template_edge_index']).astype(np.int64)
    L = int(np.asarray(inputs['LOS_batch']).reshape(-1)[0])

    A = np.zeros((36, 36), f32)
    np.add.at(A, (tei[1], tei[0]), 1.0)
    Mp = A + np.eye(36, dtype=f32)
    m72 = np.zeros((72, 72), f32)
    m72[:36, :36] = Mp.T
    m72[36:, 36:] = Mp.T

    W = {k: np.asarray(v, f32) for k, v in inputs.items()
         if k not in ('x_embedded', 'template_edge_index', 'LOS_batch')}

    sih = _pow2_scale(W['Wih'])
    shh = 1.0  # Whh rides bf16 (FWL-fast LDWEIGHTS), no scaling needed

    # one [128, .] f16 blob carries everything small, in one DMA with fat
    # descriptors (small separate DMAs starve behind the bulk queues):
    # cols 0:584 rows 0:32 = x0T | w1a; cols 584:730 rows 0:72 = m72 |
    # [eye72 | admask | dismask]; cols 730: = the sm16 slots
    vals = {
        'bc1t': W['bc1'].reshape(8, 128),
        'bhhnt': W['bhh'][2 * H:].reshape(4, 128) * shh,
        'eye8': np.eye(8, dtype=f32),
        'wc2': np.ascontiguousarray(W['Wc2'].reshape(8, 128).T),
        'eye128': np.eye(128, dtype=f32),
    }
    sA = np.zeros((32, 584), F16)
    sA[:, 0:72] = bf(x.T)
    sA[:, 72:584] = bf(W['W1a'])
    # bigB (m72 | eye+mask | sm16 slots) leads the gpsimd queue
    big16 = np.zeros((128, 74 + _SMCOLS), F16)
    big16[0:72, 0:72] = bf(m72)
    big16[0:36, 72] = 1.0
    big16[36:72, 73] = 1.0
    for name, (r0, nr, c0, ncn) in _SLOTS.items():
        big16[r0:r0 + nr, 74 + c0:74 + c0 + ncn] = bf(vals[name])

    # f32v: 0:24 combo24 [p, 2j+g] = bih[p+128j] (+ bhh for j<8); 24:28 bhh_n;
    # 28 bc2
    f32v = np.zeros((128, 29), f32)
    bih_t = W['bih'].reshape(12, 128).T
    bhh_t = W['bhh'].reshape(12, 128).T
    combo = bih_t.copy()
    combo[:, 0:8] += bhh_t[:, 0:8]
    f32v[:, 0:24:2] = combo
    f32v[:, 1:24:2] = combo
    f32v[:, 24:28] = bhh_t[:, 8:12]
    f32v[:, 28] = W['bc2'][0]

    gw16 = np.concatenate([
        _pack_kchunks_orig(W['W1b'], H), _pack_kchunks_orig(W['Wha'], H),
        _pack_kchunks_orig(W['Whb'], H)], axis=1).astype(F16)

    def q8(w, s):
        return np.clip(w * s, -15.5, 15.5).astype(F8E3)

    wiht = q8(_pack_kchunks_orig(np.ascontiguousarray(W['Wih'].T), 1536), sih)
    whht = _pack_kchunks_orig(np.ascontiguousarray(W['Whh'].T), 1536).astype(
        ml_dtypes.bfloat16)

    blobs = {
        'sA': sA,
        'big16': big16,
        'f32v': f32v,
        'whht': whht,
        'wc1': bf(_pack_kchunks_orig(W['Wc1'], 1024)),
        'gw16': gw16,
        'wiht0': np.ascontiguousarray(wiht[:, 0:4608]),
        'wiht1': np.ascontiguousarray(wiht[:, 4608:9216]),
        'wiht2': np.ascontiguousarray(wiht[:, 9216:13824]),
        'wiht3': np.ascontiguousarray(wiht[:, 13824:18432]),
    }
    return blobs, L, sih, shh


def _emit_fast(ctx, tc, d, out_dram, L, sih, shh):
    import concourse.mybir as mybir
    nc = tc.nc
    f32 = mybir.dt.float32
    f16 = mybir.dt.float16
    AF = mybir.ActivationFunctionType
    AL = mybir.AluOpType

    wts = ctx.enter_context(tc.tile_pool(name="wts", bufs=1))
    act = ctx.enter_context(tc.tile_pool(name="act", bufs=1))
    tmp = ctx.enter_context(tc.tile_pool(name="tmp", bufs=2))
    pu = ctx.enter_context(tc.tile_pool(name="pu", bufs=2, space="PSUM"))
    pvt = ctx.enter_context(tc.tile_pool(name="pvt", bufs=2, space="PSUM"))
    psm = ctx.enter_context(tc.tile_pool(name="psm", bufs=3, space="PSUM"))
    pgi = ctx.enter_context(tc.tile_pool(name="pgi", bufs=1, space="PSUM"))

    # ---- inputs -> SBUF, ordered by first use across the DMA queues ----
    sA = wts.tile([32, 584], f16, tag='sA')
    nc.sync.dma_start(sA[:, :], d['sA'])
    x0T = sA[:, 0:72]
    w1a = sA[:, 72:584]
    big16 = wts.tile([128, 74 + _SMCOLS], f16, tag='big16')
    nc.gpsimd.dma_start(big16[:, :], d['big16'])
    m72 = big16[0:72, 0:72]
    aggmask = big16[0:72, 0:NSPAN]   # [Mp.T | admask | dismask]
    masks = big16[0:72, 72:74]
    f32v = wts.tile([128, 29], f32, tag='f32v')
    nc.sync.dma_start(f32v[:, :], d['f32v'])
    # wiht3/whht/wc1 are needed late (gi drain/GRU/classifier); their
    # tiles live in thr (bufs=1) behind dummy writers keyed on GIN
    # progress, so their DMAs don't steal early bandwidth from gw16/wiht
    thr = ctx.enter_context(tc.tile_pool(name="thr", bufs=1))
    wiht3_dummy = thr.tile([1, 1], f16, tag='wiht3s')
    whht_dummy = thr.tile([1, 1], f16, tag='whht')
    wc1_dummy = thr.tile([1, 1], f16, tag='wc1')
    wiht3_tile = None

    def S(name):
        r0, nr, c0, ncn = _SLOTS[name]
        return big16[r0:r0 + nr, 74 + c0:74 + c0 + ncn]

    w1b_hi = wts.tile([128, 1024], f16, tag='w1b_hi')   # k-chunks 2,3
    w1b_lo = wts.tile([128, 1024], f16, tag='w1b_lo')   # k-chunks 0,1
    wha_t = wts.tile([128, 2048], f16, tag='wha')
    whb_t = wts.tile([128, 2048], f16, tag='whb')
    wiht_t = [wts.tile([128, 3 * 1536], mybir.dt.float8e3, tag=f'wiht{q}',
                       name=f'wiht{q}') for q in range(3)]
    nc.gpsimd.dma_start(w1b_hi[:, :], d['gw16'][:, 1024:2048])
    nc.gpsimd.dma_start(w1b_lo[:, :], d['gw16'][:, 0:1024])
    nc.gpsimd.dma_start(wha_t[:, :], d['gw16'][:, 2048:4096])
    nc.gpsimd.dma_start(whb_t[:, :], d['gw16'][:, 4096:6144])
    nc.gpsimd.dma_start(wiht_t[0][:, :], d['wiht0'])
    nc.gpsimd.dma_start(wiht_t[1][:, :], d['wiht1'])
    nc.gpsimd.dma_start(wiht_t[2][:, :], d['wiht2'])

    def wb_slice(l, fi, fo):
        if l == 0:
            t = w1b_hi if fi >= 2 else w1b_lo
            base = H * (fi % 2) + 128 * fo
            return t[:, base:base + 128]
        return whb_t[:, H * fi + 128 * fo:H * fi + 128 * fo + 128]

    def wiht_chunk(kc, j):
        q, r = divmod(kc, 3)
        base = 1536 * r + 128 * j
        t = wiht3_tile if q == 3 else wiht_t[q]
        return t[:, base:base + 128]

    # prefetch the sqrt ACT table (first LN would otherwise stall ~1.3us)
    sc1 = act.tile([1, 1], f32, tag='sc1')
    nc.vector.memset(sc1[:, :], 1.0)
    sc2 = act.tile([1, 1], f32, tag='sc2')
    eps = act.tile([72, 1], f32, tag='eps')
    nc.vector.memset(eps[:, :], LN_EPS)
    nc.scalar.activation(sc2[:, :], sc1[:, :], AF.Sqrt)

    gi_ps = pgi.tile([128, 24], f32, tag='gi')

    # ---- GIN layers ----------------------------------------------------
    # Activations live feature-major between layers as hnT [128, 4*NSPAN];
    # cols [NSPAN*c, NSPAN*c+72) are nodes, cols +72..74 the pooled feats.
    gi_backlog = []
    hnT_tiles = []
    gi_poffs = []
    hT, hcols = x0T, 32
    for l in range(3):
        wa = w1a if l == 0 else wha_t
        nk = max(hcols // 128, 1)

        # u = (Mp @ h) @ Wa.  For l>=1, hnT already carries (Mp @ h).T from
        # the previous layer's agg-folded transpose, so u comes straight
        # from those chunks; for l=0, z = x0 @ W1a then u = Mp @ z.
        if l == 0:
            z_h = [pu.tile([72, H // 2], f32, tag='pu', name=f'z{q}')
                   for q in range(2)]
            for q in range(2):
                nc.tensor.matmul(z_h[q][:, :], x0T,
                                 w1a[:, q * (H // 2):(q + 1) * (H // 2)],
                                 start=True, stop=True)
            z_sb = tmp.tile([72, H], f16, tag='z_sb')
            nc.vector.tensor_copy(z_sb[:, 0:H // 2], z_h[0][:, :])
            nc.scalar.copy(z_sb[:, H // 2:], z_h[1][:, :])
            u_h = [pu.tile([72, H // 2], f32, tag='pu', name=f'u{l}{q}')
                   for q in range(2)]
            for q in range(2):
                nc.tensor.matmul(u_h[q][:, :], m72,
                                 z_sb[:, q * (H // 2):(q + 1) * (H // 2)],
                                 start=True, stop=True)
        else:
            u_h = [pu.tile([72, H // 2], f32, tag='pu', name=f'u{l}{q}')
                   for q in range(2)]
            for ci, c in enumerate((0, 2, 1, 3)):
                for q in range(2):
                    nc.tensor.matmul(
                        u_h[q][:, :], hT[c][:, 0:72],
                        wa[:, H * c + q * (H // 2):H * c + (q + 1) * (H // 2)],
                        start=(ci == 0), stop=(ci == 3))

        # interleave layer-1's gi matmuls into layer-3's LN gap (their
        # wiht quarters have landed by then; earlier they'd stall on DMA)
        if l == 2:
            for fn in gi_backlog[:4]:
                fn()
            gi_backlog = gi_backlog[4:]

        # LN stats
        bst = tmp.tile([72, 12], f32, tag='bst')
        nc.vector.bn_stats(bst[:, 0:6], u_h[0][:, :])
        nc.vector.bn_stats(bst[:, 6:12], u_h[1][:, :])
        mv = tmp.tile([72, 2], f32, tag='mv')
        nc.vector.bn_aggr(mv[:, :], bst[:, :])
        negm = tmp.tile([72, 1], f32, tag='negm')
        nc.gpsimd.tensor_scalar(negm[:, :], mv[:, 0:1], -1.0, None, AL.mult)
        std = tmp.tile([72, 1], f32, tag='std')
        nc.scalar.activation(std[:, :], mv[:, 1:2], AF.Sqrt,
                             bias=eps[:, 0:1])
        rstd = tmp.tile([72, 1], f32, tag='rstd')
        nc.vector.reciprocal(rstd[:, :], std[:, :])
        # per-node 1/sigma folds into the transpose rhs, which also carries
        # the NEXT layer's aggregation (Mp.T) and the pooling masks; the
        # last layer only needs the pooled columns
        ncols = NSPAN if l < 2 else 2
        poff = 72 if l < 2 else 0
        dgm = tmp.tile([72, ncols], f16, tag='dgm')
        nc.vector.tensor_scalar_mul(
            dgm[:, :], aggmask if l < 2 else masks, rstd[:, 0:1])

        # r' = relu(u - m) node-major (the 1/sigma rides on dgm; relu
        # commutes with the positive scale); ACT half finishes first, so
        # its chunks (2,3) transpose first
        r_lo = tmp.tile([72, H // 2], f16, tag='r_lo')
        r_hi = tmp.tile([72, H // 2], f16, tag='r_hi')
        nc.scalar.activation(r_hi[:, :], u_h[1][:, :], AF.Relu,
                             bias=negm[:, 0:1], scale=1.0)
        us0 = tmp.tile([72, H // 2], f32, tag='us0')
        nc.vector.tensor_scalar_sub(us0[:, :], u_h[0][:, :], mv[:, 0:1])
        nc.vector.tensor_scalar_max(r_lo[:, :], us0[:, :], 0.0)

        # rT chunks: r'_chunk.T @ (rstd-scaled [Mp.T | masks]) -- the next
        # layer's aggregation and the pooling ride the transpose for free
        rT = tmp.tile([128, 4 * ncols], f16, tag='rT')
        for i, c in enumerate((2, 3, 0, 1)):
            src_r = r_hi if c >= 2 else r_lo
            tp = psm.tile([128, ncols], f32, tag='psm')
            nc.tensor.matmul(tp[:, :],
                             src_r[:, 128 * (c % 2):128 * (c % 2) + 128],
                             dgm, start=True, stop=True)
            if i % 2 == 0:
                nc.vector.tensor_copy(rT[:, ncols * c:ncols * (c + 1)], tp[:, :])
            else:
                nc.scalar.copy(rT[:, ncols * c:ncols * (c + 1)], tp[:, :])

        # vT chunks = Wb-chunk.T @ rT-chunk
        vt_ps = [pvt.tile([128, 2 * ncols], f32, tag='pvt', name=f'vt{l}{q}')
                 for q in range(2)]
        FI = (2, 3, 0, 1)
        for ki, fi in enumerate(FI):
            for fo in range(4):
                q, o = fo % 2, fo // 2
                nc.tensor.matmul(
                    vt_ps[q][:, ncols * o:ncols * (o + 1)],
                    wb_slice(l, fi, fo),
                    rT[:, ncols * fi:ncols * (fi + 1)],
                    start=(ki == 0 and fo < 2), stop=(ki == 3),
                    skip_group_check=True)
        hnT = [act.tile([128, ncols], f16, tag=f'hnT{l}{fo}',
                        name=f'hnT{l}{fo}') for fo in range(4)]
        for fo in range(4):
            q, o = fo % 2, fo // 2
            srcp = vt_ps[q][:, ncols * o:ncols * (o + 1)]
            if fo < 2:
                nc.vector.tensor_copy(hnT[fo][:, :], srcp)
            else:
                nc.scalar.copy(hnT[fo][:, :], srcp)
        hnT_tiles.append(hnT)
        gi_poffs.append(poff)

        # queue this layer's gi matmuls; rhs = the pooled cols of hnT
        def make_gi(lv, kcv, mcv):
            def emit_gi():
                src = hnT_tiles[lv][mcv]
                po = gi_poffs[lv]
                for j in range(12):
                    nc.tensor.matmul(
                        gi_ps[:, 2 * j:2 * j + 2],
                        wiht_chunk(kcv, j),
                        src[:, po:po + 2],
                        start=(kcv == 0 and j == 0), stop=(kcv == 11),
                        skip_group_check=True)
            return emit_gi
        for mc in range(4):
            gi_backlog.append(make_gi(l, 4 * l + mc, mc))
        if l == 0:
            nc.vector.tensor_copy(wiht3_dummy[0:1, 0:1], hnT[0][0:1, 0:1])
            wiht3_tile = thr.tile([128, 3 * 1536], mybir.dt.float8e3,
                                  tag='wiht3s', name='wiht3s')
            nc.sync.dma_start(wiht3_tile[:, :], d['wiht3'])
            nc.vector.tensor_copy(whht_dummy[0:1, 0:1], hnT[0][0:1, 0:1])
            whht = thr.tile([128, 4 * 1536], mybir.dt.bfloat16, tag='whht')
            nc.sync.dma_start(whht[:, :], d['whht'])
        elif l == 2:
            nc.vector.tensor_copy(wc1_dummy[0:1, 0:1], hnT[0][0:1, 0:1])
            wc1 = thr.tile([128, 4 * 1024], f16, tag='wc1')
            nc.sync.dma_start(wc1[:, :], d['wc1'])
        hT = hnT
        hcols = H

    for fn in gi_backlog:
        fn()

    # switch ACT to the sigmoid/tanh table while PE drains gi matmuls;
    # reading layer-3 output pins this after the last LN sqrt
    nc.scalar.activation(sc2[:, :], hnT_tiles[2][0][0:1, 0:1], AF.Sigmoid)

    # ---- GRU setup + step 0 (h=0 so gr=0; gates come from gib2) ----
    gib2 = act.tile([128, 24], f32, tag='gib2')
    nc.vector.scalar_tensor_tensor(gib2[:, :], gi_ps[:, :], 1.0 / sih,
                                   f32v[:, 0:24], AL.mult, AL.add)
    g0 = 0 if L > 1 else 1
    w = tmp.tile([128, 4], f32, tag='w')
    nc.scalar.activation(w[:, :], gib2[:, 8 + g0:16:2], AF.Sigmoid,
                         scale=-1.0)
    n = tmp.tile([128, 4], f32, tag='n')
    nc.scalar.activation(n[:, :], gib2[:, 16 + g0::2], AF.Tanh)
    # h state as two half-tiles so the next step's first matmuls start as
    # soon as the first half lands
    h01 = tmp.tile([128, 2], f16, tag='h01')
    nc.vector.tensor_tensor(h01[:, :], w[:, 0:2], n[:, 0:2], AL.mult)
    h23 = tmp.tile([128, 2], f16, tag='h23')
    nc.gpsimd.tensor_tensor(h23[:, :], w[:, 2:4], n[:, 2:4], AL.mult)

    # r/z bias closers for steps >= 1 (only needed after step 0's gates)
    gibT = []  # per graph: ([4,128] r-bias, [4,128] z-bias) fp16
    for g in range(2):
        pair = []
        for half in range(2):
            gb = tmp.tile([128, 4], f16, tag='gib_h')
            nc.vector.tensor_copy(gb[:, :], gib2[:, 8 * half + g:8 * half + 8:2])
            tp = psm.tile([4, 128], f16, tag='psm')
            nc.tensor.transpose(tp[:, :], gb[:, :], S('eye128'))
            t = act.tile([4, 128], f16, tag=f'gibT{g}{half}')
            nc.scalar.activation(t[:, :], tp[:, :], AF.Identity, scale=shh)
            pair.append(t)
        gibT.append(pair)

    def hcol(c):
        return (h01 if c < 2 else h23)[:, c % 2:c % 2 + 1]

    eye4 = S('eye8')[0:4, 0:4]
    for t in range(1, L):
        gs = 0 if t < L - 1 else 1
        grr = psm.tile([128, 4], f32, tag='psm')
        grn = psm.tile([128, 4], f32, tag='psm')
        gzz = psm.tile([128, 4], f32, tag='psm')
        for j in range(4):
            for c in range(4):
                nc.tensor.matmul(
                    grr[:, j:j + 1],
                    whht[:, 1536 * c + 128 * j:1536 * c + 128 * (j + 1)],
                    hcol(c), start=(c == 0 and j == 0),
                    stop=False, skip_group_check=True)
        nc.tensor.matmul(grr[:, :], gibT[gs][0][:, :], eye4,
                         start=False, stop=True, skip_group_check=True)
        for j in range(8, 12):
            for c in range(4):
                nc.tensor.matmul(
                    grn[:, j - 8:j - 7],
                    whht[:, 1536 * c + 128 * j:1536 * c + 128 * (j + 1)],
                    hcol(c), start=(c == 0 and j == 8),
                    stop=(c == 3 and j == 11), skip_group_check=True)
        for j in range(4, 8):
            for c in range(4):
                nc.tensor.matmul(
                    gzz[:, j - 4:j - 3],
                    whht[:, 1536 * c + 128 * j:1536 * c + 128 * (j + 1)],
                    hcol(c), start=(c == 0 and j == 4),
                    stop=False, skip_group_check=True)
        nc.tensor.matmul(gzz[:, :], gibT[gs][1][:, :], eye4,
                         start=False, stop=True, skip_group_check=True)

        r = tmp.tile([128, 4], f32, tag='r')
        nc.scalar.activation(r[:, :], grr[:, :], AF.Sigmoid,
                             scale=1.0 / shh)
        nt = tmp.tile([128, 4], f32, tag='nt')
        nc.vector.scalar_tensor_tensor(nt[:, :], grn[:, :], 1.0 / shh,
                                       r[:, :], AL.mult, AL.mult)
        nc.vector.tensor_tensor(nt[:, :], nt[:, :], gib2[:, 16 + gs::2],
                                AL.add)
        n = tmp.tile([128, 4], f32, tag='n')
        nc.scalar.activation(n[:, :], nt[:, :], AF.Tanh)
        # w = 1 - z via sigmoid(-x); h_new = h + w * (n - h)
        w = tmp.tile([128, 4], f32, tag='w')
        nc.scalar.activation(w[:, :], gzz[:, :], AF.Sigmoid,
                             scale=-1.0 / shh)
        d = tmp.tile([128, 4], f32, tag='d')
        nc.vector.tensor_tensor(d[:, 0:2], n[:, 0:2], h01[:, :],
                                AL.subtract)
        nc.vector.tensor_tensor(d[:, 2:4], n[:, 2:4], h23[:, :],
                                AL.subtract)
        wd = tmp.tile([128, 4], f32, tag='wd')
        nc.vector.tensor_tensor(wd[:, :], w[:, :], d[:, :], AL.mult)
        nh01 = tmp.tile([128, 2], f16, tag='h01')
        nc.vector.tensor_tensor(nh01[:, :], h01[:, :], wd[:, 0:2], AL.add)
        nh23 = tmp.tile([128, 2], f16, tag='h23')
        nc.gpsimd.tensor_tensor(nh23[:, :], h23[:, :], wd[:, 2:4], AL.add)
        h01, h23 = nh01, nh23

    # ---- classifier ----
    hid_ps = psm.tile([128, 8], f32, tag='psm')
    for mc in range(8):
        for c in range(4):
            nc.tensor.matmul(
                hid_ps[:, mc:mc + 1],
                wc1[:, 1024 * c + 128 * mc:1024 * c + 128 * (mc + 1)],
                hcol(c), start=(c == 0 and mc == 0),
                stop=(c == 3 and mc == 7), skip_group_check=True)
    hid = tmp.tile([128, 8], f16, tag='hid_sb')
    nc.scalar.activation(hid[:, :], hid_ps[:, :], AF.Relu)
    fin_ps = psm.tile([1, 1], f32, tag='psm')
    for mc in range(8):
        nc.tensor.matmul(fin_ps[:, :], hid[:, mc:mc + 1], S('wc2')[:, mc:mc + 1],
                         start=(mc == 0), stop=(mc == 7))
    out_sb = tmp.tile([1, 1], f32, tag='out_sb')
    nc.scalar.activation(out_sb[:, :], fin_ps[:, :], AF.Identity,
                         bias=f32v[0:1, 28:29], scale=1.0)
    nc.sync.dma_start(out_dram, out_sb[:, :], single_packet=True)


# ------------------------------------------------------------ general path
# (previous fully-general emission; used when biases/LN affine are nonzero)

_XSLOTS = {}
_XCOLS = 0


def _xslot(name, nrows, ncols):
    global _XCOLS
    _XSLOTS[name] = (nrows, _XCOLS, ncols)
    _XCOLS += ncols


_xslot('x0', 72, 32)
_xslot('eye72', 72, 72)
_xslot('w1a', 32, H)
_xslot('m72', 72, 72)
_xslot('ones72', 1, 72)
_xslot('brows', 1, 4 * H + 2)

_GSLOTS = {}
_GSMCOLS = 0


def _gslot(name, nrows, ncols):
    global _GSMCOLS
    _GSLOTS[name] = (0, nrows, _GSMCOLS, ncols)
    _GSMCOLS += ncols


_gslot('bc1t', 8, 128)
_gslot('bhhnt', 4, 128)
_gslot('eye8', 8, 8)
_gslot('wc2', 128, 8)
_gslot('eye128', 128, 128)


def _prep_general(inputs):
    f32 = np.float32

    def bf(x):
        return np.asarray(x, f32).astype(F16)

    x = np.asarray(inputs['x_embedded'], f32)
    tei = np.asarray(inputs['template_edge_index']).astype(np.int64)
    L = int(np.asarray(inputs['LOS_batch']).reshape(-1)[0])

    A = np.zeros((36, 36), f32)
    np.add.at(A, (tei[1], tei[0]), 1.0)
    Mp = A + np.eye(36, dtype=f32)
    m72 = np.zeros((72, 72), f32)
    m72[:36, :36] = Mp.T
    m72[36:, 36:] = Mp.T

    W = {k: np.asarray(v, f32) for k, v in inputs.items()
         if k not in ('x_embedded', 'template_edge_index', 'LOS_batch')}

    xvals = {
        'x0': x,
        'eye72': np.eye(72, dtype=f32),
        'w1a': W['W1a'],
        'm72': m72,
        'ones72': np.ones((1, 72), f32),
        'brows': np.concatenate(
            [W['b1a'], W['b1b'], W['bha'], W['bhb'], [0.0], [0.0]]
        ).reshape(1, 4 * H + 2),
    }
    xe16 = np.zeros((72, _XCOLS), F16)
    for name, (nr, c0, ncn) in _XSLOTS.items():
        xe16[0:nr, c0:c0 + ncn] = bf(xvals[name])

    vals = {
        'bc1t': W['bc1'].reshape(8, 128),
        'bhhnt': W['bhh'][2 * H:].reshape(4, 128),
        'eye8': np.eye(8, dtype=f32),
        'wc2': np.ascontiguousarray(W['Wc2'].reshape(8, 128).T),
        'eye128': np.eye(128, dtype=f32),
    }
    sm16 = np.zeros((128, _GSMCOLS), F16)
    for name, (r0, nr, c0, ncn) in _GSLOTS.items():
        sm16[r0:r0 + nr, c0:c0 + ncn] = bf(vals[name])

    f32v = np.zeros((128, 53), f32)
    bih_t = W['bih'].reshape(12, 128).T
    bhh_t = W['bhh'].reshape(12, 128).T
    combo = bih_t.copy()
    combo[:, 0:8] += bhh_t[:, 0:8]
    f32v[:, 0:24:2] = combo
    f32v[:, 1:24:2] = combo
    f32v[:, 24:28] = bhh_t[:, 8:12]
    f32v[:, 28] = W['bc2'][0]
    f32v[:, 29:33] = W['g1'].reshape(4, 128).T
    f32v[:, 33:37] = W['be1'].reshape(4, 128).T
    f32v[:, 37:41] = W['gh'].reshape(4, 128).T
    f32v[:, 41:45] = W['beh'].reshape(4, 128).T
    f32v[:, 45:49] = W['b1b'].reshape(4, 128).T
    f32v[:, 49:53] = W['bhb'].reshape(4, 128).T

    gw16 = np.concatenate([
        _pack_kchunks_orig(W['W1b'], H), _pack_kchunks_orig(W['Wha'], H),
        _pack_kchunks_orig(W['Whb'], H)], axis=1).astype(F16)

    blobs = {
        'xe0': xe16[:, 0:104].copy(),
        'xe16': xe16,
        'sm16': sm16,
        'gw16': gw16,
        'f32v': f32v,
        'wiht': bf(_pack_kchunks_orig(np.ascontiguousarray(W['Wih'].T), 1536)),
        'whht': bf(_pack_kchunks_orig(np.ascontiguousarray(W['Whh'].T), 1536)),
        'wc1': bf(_pack_kchunks_orig(W['Wc1'], 1024)),
    }
    return blobs, L


def _emit_general(ctx, tc, d, out_dram, L):
    import concourse.mybir as mybir
    nc = tc.nc
    f32 = mybir.dt.float32
    f16 = mybir.dt.float16
    AF = mybir.ActivationFunctionType
    AL = mybir.AluOpType

    wts = ctx.enter_context(tc.tile_pool(name="wts", bufs=1))
    act = ctx.enter_context(tc.tile_pool(name="act", bufs=1))
    tmp = ctx.enter_context(tc.tile_pool(name="tmp", bufs=2))
    pu = ctx.enter_context(tc.tile_pool(name="pu", bufs=2, space="PSUM"))
    pvt = ctx.enter_context(tc.tile_pool(name="pvt", bufs=2, space="PSUM"))
    psm = ctx.enter_context(tc.tile_pool(name="psm", bufs=3, space="PSUM"))
    pgi = ctx.enter_context(tc.tile_pool(name="pgi", bufs=1, space="PSUM"))

    xe0 = wts.tile([72, 104], f16, tag='xe0')
    nc.sync.dma_start(xe0[:, :], d['xe0'])
    x0s = xe0[:, 0:32]
    eye72 = xe0[:, 32:104]
    xe16 = wts.tile([72, _XCOLS], f16, tag='xe16')
    nc.sync.dma_start(xe16[:, :], d['xe16'])

    def X(name):
        nr, c0, ncn = _XSLOTS[name]
        return xe16[0:nr, c0:c0 + ncn]

    sm16 = wts.tile([128, _GSMCOLS], f16, tag='sm16')
    nc.sync.dma_start(sm16[:, :], d['sm16'])
    f32v = wts.tile([128, 53], f32, tag='f32v')
    nc.sync.dma_start(f32v[:, :], d['f32v'])
    whht = wts.tile([128, 4 * 1536], f16, tag='whht')
    nc.sync.dma_start(whht[:, :], d['whht'])
    wc1 = wts.tile([128, 4 * 1024], f16, tag='wc1')
    nc.sync.dma_start(wc1[:, :], d['wc1'])

    def S(name):
        r0, nr, c0, ncn = _GSLOTS[name]
        return sm16[r0:r0 + nr, c0:c0 + ncn]

    gw16 = wts.tile([128, 3 * 4 * H], f16, tag='gw16')
    wiht_t = [wts.tile([128, 3 * 1536], f16, tag=f'wiht{q}',
                       name=f'wiht{q}') for q in range(4)]
    nc.gpsimd.dma_start(gw16[:, 0:2048], d['gw16'][:, 0:2048])
    nc.gpsimd.dma_start(wiht_t[0][:, :], d['wiht'][:, 0:4608])
    nc.gpsimd.dma_start(gw16[:, 2048:4096], d['gw16'][:, 2048:4096])
    nc.gpsimd.dma_start(wiht_t[1][:, :], d['wiht'][:, 4608:9216])
    nc.gpsimd.dma_start(gw16[:, 4096:6144], d['gw16'][:, 4096:6144])
    nc.gpsimd.dma_start(wiht_t[2][:, :], d['wiht'][:, 9216:13824])
    nc.gpsimd.dma_start(wiht_t[3][:, :], d['wiht'][:, 13824:18432])

    def wiht_chunk(kc, j):
        q, r = divmod(kc, 3)
        base = 1536 * r + 128 * j
        return wiht_t[q][:, base:base + 128]

    sc1 = act.tile([1, 1], f32, tag='sc1')
    nc.vector.memset(sc1[:, :], 1.0)
    sc2 = act.tile([1, 1], f32, tag='sc2')
    eps = act.tile([72, 1], f32, tag='eps')
    nc.vector.memset(eps[:, :], LN_EPS)
    nc.scalar.activation(sc2[:, :], sc1[:, :], AF.Sqrt)

    featsT = act.tile([128, 24], f16, tag='featsT')
    gi_ps = pgi.tile([128, 24], f32, tag='gi')

    x0T = tmp.tile([32, 72], f16, tag='x0T')
    tp0 = psm.tile([128, 72], f16, tag='psm')
    nc.tensor.transpose(tp0[0:32, :], x0s, eye72)
    nc.vector.tensor_copy(x0T[:, :], tp0[0:32, :])

    gi_backlog = []
    hT = x0T
    hcols = 32
    for l in range(3):
        wa = X('w1a') if l == 0 else gw16[:, 2048:4096]
        wb = gw16[:, 0:2048] if l == 0 else gw16[:, 4096:6144]
        ba_off = 0 if l == 0 else 2 * H
        gcol = 29 if l == 0 else 37
        becol = 33 if l == 0 else 41
        bbtcol = 45 if l == 0 else 49
        nk = max(hcols // 128, 1)

        z_h = [pbig.tile([72, H // 2], f32, tag='pbig', name=f'z{q}')
               for q in range(2)]
        for c in range(nk):
            cs = min(128, hcols - 128 * c)
            for q in range(2):
                rhs = (wa if l == 0 else wa[:, H * c:H * (c + 1)])[
                    :, q * (H // 2):(q + 1) * (H // 2)]
                nc.tensor.matmul(z_h[q][:, :],
                                 hT[0:cs, 72 * c:72 * (c + 1)], rhs,
                                 start=(c == 0), stop=(c == nk - 1))
        z_sb = tmp.tile([72, H], f16, tag='z_sb')
        nc.vector.tensor_copy(z_sb[:, 0:H // 2], z_h[0][:, :])
        nc.scalar.copy(z_sb[:, H // 2:], z_h[1][:, :])

        u_h = [pbig.tile([72, H // 2], f32, tag='pbig', name=f'u{q}')
               for q in range(2)]
        for q in range(2):
            nc.tensor.matmul(u_h[q][:, :], X('m72'),
                             z_sb[:, q * (H // 2):(q + 1) * (H // 2)],
                             start=True, stop=False)
        for q in range(2):
            off = ba_off + q * (H // 2)
            nc.tensor.matmul(u_h[q][:, :], X('ones72'),
                             X('brows')[:, off:off + H // 2],
                             start=False, stop=True)

        bst = tmp.tile([72, 12], f32, tag='bst')
        nc.vector.bn_stats(bst[:, 0:6], u_h[0][:, :])
        nc.vector.bn_stats(bst[:, 6:12], u_h[1][:, :])
        mv = tmp.tile([72, 2], f32, tag='mv')
        nc.vector.bn_aggr(mv[:, :], bst[:, :])
        std = tmp.tile([72, 1], f32, tag='std')
        nc.scalar.activation(std[:, :], mv[:, 1:2], AF.Sqrt,
                             bias=eps[:, 0:1])
        rstd = tmp.tile([72, 1], f32, tag='rstd')
        nc.vector.reciprocal(rstd[:, :], std[:, :])
        mb = tmp.tile([72, 1], f32, tag='mb')
        nc.vector.scalar_tensor_tensor(mb[:, :], mv[:, 0:1], -1.0,
                                       rstd[:, 0:1], AL.mult, AL.mult)

        us = tmp.tile([72, H], f16, tag='us')
        nc.vector.tensor_scalar(us[:, 0:H // 2], u_h[0][:, :],
                                mv[:, 0:1], rstd[:, 0:1],
                                AL.subtract, AL.mult)
        nc.scalar.activation(us[:, H // 2:], u_h[1][:, :], AF.Identity,
                             bias=mb[:, 0:1], scale=rstd[:, 0:1])
        rT = tmp.tile([128, 4 * 72], f16, tag='rT')
        for c in range(4):
            tp = psm.tile([128, 72], f16, tag='psm')
            nc.tensor.transpose(tp[:, :], us[:, 128 * c:128 * (c + 1)],
                                eye72)
            nc.scalar.activation(rT[:, 72 * c:72 * (c + 1)], tp[:, :], AF.Relu,
                                 bias=f32v[:, becol + c:becol + c + 1],
                                 scale=f32v[:, gcol + c:gcol + c + 1])

        vt_ps = [pbig.tile([128, 2 * 72], f32, tag='pvt', name=f'vt{q}')
                 for q in range(2)]
        for fi in range(4):
            for fo in range(4):
                q, o = fo % 2, fo // 2
                nc.tensor.matmul(
                    vt_ps[q][:, 72 * o:72 * (o + 1)],
                    wb[:, H * fi + 128 * fo:H * fi + 128 * fo + 128],
                    rT[:, 72 * fi:72 * (fi + 1)],
                    start=(fi == 0 and fo < 2), stop=(fi == 3),
                    skip_group_check=True)
        hnT = tmp.tile([128, 4 * 72], f16, tag='hnT')
        for fo in range(4):
            q, o = fo % 2, fo // 2
            dst = hnT[:, 72 * fo:72 * (fo + 1)]
            srcp = vt_ps[q][:, 72 * o:72 * (o + 1)]
            bb = f32v[:, bbtcol + fo:bbtcol + fo + 1]
            if fo < 2:
                nc.vector.tensor_scalar_add(dst, srcp, bb[:, 0:1])
            else:
                nc.scalar.activation(dst, srcp, AF.Identity, bias=bb[:, 0:1])

        pf = tmp.tile([128, 8], f32, tag='pf')
        for fo in range(4):
            for g in range(2):
                nc.vector.tensor_reduce(
                    pf[:, 2 * fo + g:2 * fo + g + 1],
                    hnT[:, 72 * fo + 36 * g:72 * fo + 36 * g + 36],
                    mybir.AxisListType.X, AL.add)
        nc.vector.tensor_copy(featsT[:, 8 * l:8 * l + 8], pf[:, :])

        def make_gi(kcv):
            def emit_gi():
                for j in range(12):
                    nc.tensor.matmul(
                        gi_ps[:, 2 * j:2 * j + 2],
                        wiht_chunk(kcv, j),
                        featsT[:, 2 * kcv:2 * kcv + 2],
                        start=(kcv == 0 and j == 0), stop=(kcv == 11),
                        skip_group_check=True)
            return emit_gi
        for mc in range(4):
            gi_backlog.append(make_gi(4 * l + mc))
        hT = hnT
        hcols = H

    for kc in range(12):
        gi_backlog[kc]()
    gi_backlog = []

    gib2 = act.tile([128, 24], f32, tag='gib2')
    nc.vector.tensor_tensor(gib2[:, :], gi_ps[:, :], f32v[:, 0:24], AL.add)
    gibT = []
    for g in range(2):
        pair = []
        for half in range(2):
            gb = tmp.tile([128, 4], f16, tag='gib_h')
            nc.vector.tensor_copy(gb[:, :], gib2[:, g + 8 * half:g + 8 * half + 8:2])
            tp = psm.tile([4, 128], f16, tag='psm')
            nc.tensor.transpose(tp[:, :], gb[:, :], S('eye128'))
            t = act.tile([4, 128], f16, tag=f'gibT{g}{half}')
            nc.vector.tensor_copy(t[:, :], tp[:, :])
            pair.append(t)
        gibT.append(pair)

    g0 = 0 if L > 1 else 1
    rz = tmp.tile([128, 8], f32, tag='rz')
    nc.scalar.activation(rz[:, :], gib2[:, g0:16:2], AF.Sigmoid)
    nt = tmp.tile([128, 4], f32, tag='nt')
    nc.vector.tensor_tensor(nt[:, :], rz[:, 0:4], f32v[:, 24:28], AL.mult)
    nc.vector.tensor_tensor(nt[:, :], nt[:, :], gib2[:, 16 + g0::2], AL.add)
    n = tmp.tile([128, 4], f32, tag='n')
    nc.scalar.activation(n[:, :], nt[:, :], AF.Tanh)
    w = tmp.tile([128, 4], f32, tag='w')
    nc.gpsimd.tensor_scalar(w[:, :], rz[:, 4:8], -1.0, 1.0, AL.mult, AL.add)
    h_f = tmp.tile([128, 4], f32, tag='h_f')
    nc.gpsimd.tensor_tensor(h_f[:, :], w[:, :], n[:, :], AL.mult)
    h_b = tmp.tile([128, 4], f16, tag='h_b')
    nc.vector.tensor_tensor(h_b[:, :], w[:, :], n[:, :], AL.mult)

    eye4 = S('eye8')[0:4, 0:4]
    for t in range(1, L):
        gs = 0 if t < L - 1 else 1
        grr = psm.tile([128, 4], f32, tag='psm')
        grn = psm.tile([128, 4], f32, tag='psm')
        grz = psm.tile([128, 4], f32, tag='psm')
        for out_ps, js, closer in (
                (grr, range(0, 4), (gibT[gs][0][:, :], eye4)),
                (grn, range(8, 12), (S('bhhnt'), eye4)),
                (grz, range(4, 8), (gibT[gs][1][:, :], eye4))):
            j0 = js[0]
            for j in js:
                for c in range(4):
                    nc.tensor.matmul(
                        out_ps[:, j - j0:j - j0 + 1],
                        whht[:, 1536 * c + 128 * j:1536 * c + 128 * (j + 1)],
                        h_b[:, c:c + 1], start=(c == 0 and j == j0),
                        stop=False, skip_group_check=True)
            nc.tensor.matmul(out_ps[:, :], closer[0], closer[1],
                             start=False, stop=True, skip_group_check=True)

        r = tmp.tile([128, 4], f32, tag='r')
        nc.scalar.activation(r[:, :], grr[:, :], AF.Sigmoid)
        nt = tmp.tile([128, 4], f32, tag='nt')
        nc.vector.tensor_tensor(nt[:, :], r[:, :], grn[:, :], AL.mult)
        nc.vector.tensor_tensor(nt[:, :], nt[:, :], gib2[:, 16 + gs::2],
                                AL.add)
        n = tmp.tile([128, 4], f32, tag='n')
        nc.scalar.activation(n[:, :], nt[:, :], AF.Tanh)
        z = tmp.tile([128, 4], f32, tag='z')
        nc.scalar.activation(z[:, :], grz[:, :], AF.Sigmoid)
        zh = tmp.tile([128, 4], f32, tag='zh')
        nc.gpsimd.tensor_tensor(zh[:, :], z[:, :], h_f[:, :], AL.mult)
        w = tmp.tile([128, 4], f32, tag='w')
        nc.gpsimd.tensor_scalar(w[:, :], z[:, :], -1.0, 1.0, AL.mult, AL.add)
        wn = tmp.tile([128, 4], f32, tag='wn')
        nc.vector.tensor_tensor(wn[:, :], w[:, :], n[:, :], AL.mult)
        h_f = tmp.tile([128, 4], f32, tag='h_f')
        nc.gpsimd.tensor_tensor(h_f[:, :], wn[:, :], zh[:, :], AL.add)
        h_b = tmp.tile([128, 4], f16, tag='h_b')
        nc.vector.tensor_tensor(h_b[:, :], wn[:, :], zh[:, :], AL.add)

    hid_ps = psm.tile([128, 8], f32, tag='psm')
    for mc in range(8):
        for c in range(4):
            nc.tensor.matmul(
                hid_ps[:, mc:mc + 1],
                wc1[:, 1024 * c + 128 * mc:1024 * c + 128 * (mc + 1)],
                h_b[:, c:c + 1], start=(c == 0 and mc == 0), stop=False,
                skip_group_check=True)
    nc.tensor.matmul(hid_ps[:, :], S('bc1t'), S('eye8'),
                     start=False, stop=True, skip_group_check=True)
    hid = tmp.tile([128, 8], f16, tag='hid_sb')
    nc.scalar.activation(hid[:, :], hid_ps[:, :], AF.Relu)
    fin_ps = psm.tile([1, 1], f32, tag='psm')
    for mc in range(8):
        nc.tensor.matmul(fin_ps[:, :], hid[:, mc:mc + 1], S('wc2')[:, mc:mc + 1],
                         start=(mc == 0), stop=(mc == 7))
    out_sb = tmp.tile([1, 1], f32, tag='out_sb')
    nc.scalar.activation(out_sb[:, :], fin_ps[:, :], AF.Identity,
                         bias=f32v[0:1, 28:29], scale=1.0)
    nc.sync.dma_start(out_dram, out_sb[:, :])


# --------------------------------------------------------------- plumbing

def _is_fast_path(inputs):
    f32 = np.float32
    zeros = ['b1a', 'be1', 'b1b', 'bha', 'beh', 'bhb', 'bhh', 'bc1']
    ones = ['g1', 'gh']
    for k in zeros:
        if np.any(np.asarray(inputs[k], f32) != 0.0):
            return False
    for k in ones:
        if np.any(np.asarray(inputs[k], f32) != 1.0):
            return False
    return True


def _build_program(key, blobs, emit_fn):
    from contextlib import ExitStack
    import concourse.bacc as bacc
    import concourse.tile as tile
    import concourse.mybir as mybir

    nc = bacc.Bacc("TRN2", target_bir_lowering=False, debug=False,
                   num_devices=8)
    d = {}
    for name, arr in blobs.items():
        d[name] = nc.dram_tensor(name, list(arr.shape),
                                 mybir.dt.from_np(arr.dtype),
                                 kind="ExternalInput").ap()
    out_dram = nc.dram_tensor("out", [1], mybir.dt.float32,
                              kind="ExternalOutput").ap()
    with tile.TileContext(nc) as tc:
        with ExitStack() as ctx:
            emit_fn(ctx, tc, d, out_dram)
    nc.compile()
    return nc


def _install_ntff_hook():
    """The agent image's antenv lacks axon_hooks; recreate it so
    run_bass_kernel_spmd(trace=True) can capture NTFF profiles."""
    import sys, types
    try:
        import antenv
        if 'antenv.axon_hooks' in sys.modules:
            return
        mod = types.ModuleType('antenv.axon_hooks')
        mod._hook = None

        def set_axon_ntff_profile_hook(hk):
            mod._hook = hk

        def get_axon_ntff_profile_hook():
            return mod._hook

        mod.set_axon_ntff_profile_hook = set_axon_ntff_profile_hook
        mod.get_axon_ntff_profile_hook = get_axon_ntff_profile_hook
        sys.modules['antenv.axon_hooks'] = mod
        antenv.axon_hooks = mod
        from trn_agent_boot.trn_boot import _ntff_profile_via_ctypes
        so = '/opt/axon/libaxon_pjrt.so'
        if os.path.exists(so):
            mod._hook = _ntff_profile_via_ctypes(so)
    except Exception as e:  # profiling is best-effort
        print(f"ntff hook install failed: {e}")


def kernel(**inputs):
    from concourse.bass_utils import run_bass_kernel_spmd

    fast = _is_fast_path(inputs)
    if fast:
        blobs, L, sih, shh = _prep_fast(inputs)
        key = ('fast', L)
        if key not in _prog_cache:
            _prog_cache[key] = _build_program(
                key, blobs,
                lambda ctx, tc, d, o: _emit_fast(ctx, tc, d, o, L, sih, shh))
    else:
        blobs, L = _prep_general(inputs)
        key = ('gen', L)
        if key not in _prog_cache:
            _prog_cache[key] = _build_program(
                key, blobs,
                lambda ctx, tc, d, o: _emit_general(ctx, tc, d, o, L))
    nc = _prog_cache[key]

    in_maps = [dict(blobs) for _ in range(8)]
    trace = bool(int(os.environ.get('KERNEL_TRACE', '0')))
    if trace:
        _install_ntff_hook()
    res = run_bass_kernel_spmd(nc, in_maps, list(range(8)), trace=trace)
    last_run_info['exec_time_ns'] = res.exec_time_ns
    last_run_info['results'] = res
    return np.asarray(res.results[0]['out'], np.float32).reshape(1)
